# revision 14
# baseline (speedup 1.0000x reference)
# MixGAT layer (GATConv + beta-mix swish) on 8 Trainium2 NeuronCores, v2.
#
# Strategy (dst-node sharding):
#  - Nodes partitioned across 8 cores by dst id; each core owns N/8 dst rows.
#  - KEY CHANGE vs v1: aggregation is linear in xp = x @ W, so we aggregate
#    RAW x features per dst and project ONCE per dst node afterwards:
#      out[d] = (softmax-weighted-sum_e x[src_e]) / denom @ W
#    This removes the on-device projection-table build (old phase A): the
#    gather table is just x cast to f16 on the host (node-major, 256B rows
#    instead of 512B).
#  - Launch 1 (tiny): per-node attention stats a_src/a_dst = x @ (W@att) on
#    device; host expands them into per-edge streams (indexing only).
#  - Launch 2, per superblock (128 dst nodes = 4 fixed 32-node groups):
#      dma_gather x rows per edge slot (lo/hi int16-index split, 256B rows,
#      single-packet descriptors spread over 4 SWDGE queues),
#      expa = exp(lrelu(a_src+a_dst)) from streams,
#      Mw[e, h*32+c] = expa[e, h] * onehot(dst slot c),
#      per 128-edge block:  U2[feat, slot] += glo_blk(lhsT) @ Mw(rhs)
#                           Dt[slot, 1]   += Mw(lhsT) @ ones(rhs)
#      per group: project  Z[c, h*32+o] = U2[:, h*32..](lhsT) @ W[:, h*32..]
#      per sb: denominators to [c, g, h] via 4 partition-shifted copies,
#      normalize, beta-mix swish, ONE node-ordered output DMA.
#    Fixed 32-node groups keep outputs contiguous: no scratch roundtrip and
#    no permutation pass. Per-group block counts are padded to the max over
#    cores so one SPMD module serves all 8 cores.
#
# kernel(**inputs) is self-contained: preprocessing is pure numpy (sorting /
# indexing / dtype casts only), device kernels built with bass/Tile, run via
# run_bass_kernel_spmd on cores 0-7.

import numpy as np

import concourse.bass as bass
import concourse.mybir as mybir
import concourse.tile as tile
from concourse import bacc
from concourse.bass_utils import run_bass_kernel_spmd

F32 = mybir.dt.float32
F16 = mybir.dt.float16
I16 = mybir.dt.int16

# problem constants
N_NODES = 50000
IN_DIM = 128
HEADS = 4
OUT_DIM = 32
LEAKY_SLOPE = 0.2
BETA = 0.5
CMIX = 1.2
N_CORES = 8

# static schedule constants
WIN = 32          # dst nodes per group (PSUM slots = HEADS*WIN = 128)
BLK = 128         # edges per block (gather slots -> partitions)
GPB = 4           # groups per superblock (4*32 = 128 dst nodes)
SPLIT = 32768     # int16-addressable table split
DEAD = 100.0      # colidx value for dead slots (never equals iota 0..31)
GNJ = 1024        # rows per dma_gather call (SWDGE ring holds scratch/16
                  # descriptors; stay strictly under that at 1 desc/row)
NQ = 4            # SWDGE queues to spread gathers over
SCRATCH = 32768   # dynamic dma scratch (ring) bytes per partition
SINGLE_PACKET = True


def _wrap16(v):
    """idx vector [S*16] -> dma_gather idx layout [128, S]."""
    s = v.reshape(-1, 16).T                      # [16, S]
    return np.tile(s, (8, 1)).astype(np.int16)   # [128, S]


def _gather_chunks(total, gnj):
    out = []
    o = 0
    while o < total:
        c = min(gnj, total - o)
        out.append((o, c))
        o += c
    return out


class Cfg:
    def __init__(self, npc, n_cores=N_CORES, bias_nonzero=False, repeat=1,
                 blevel=4, gnj=GNJ, nq=NQ, scratch=SCRATCH, sp=SINGLE_PACKET,
                 poolpct=30):
        self.npc = npc
        self.n_cores = n_cores
        self.bias_nonzero = bias_nonzero
        self.repeat = repeat
        self.blevel = blevel   # 1 gather only; 2 +mw; 3 +matmul; 4 full
        self.gnj = gnj
        self.nq = nq
        self.scratch = scratch
        self.sp = sp
        self.poolpct = poolpct  # % of oneh/mw blocks built on GPSIMD


# ---------------------------------------------------------------- host side

def build_nc_stats(n_rows, n_cores, repeat=1):
    """Launch-1 mini kernel: statv[8, n_rows] = (W@[as|ad]).T @ xT_slab."""
    nc = bacc.Bacc("TRN2", target_bir_lowering=False, debug=False,
                   num_devices=n_cores)
    TW = 512
    H2 = 2 * HEADS
    xs_t = nc.dram_tensor("xT_slab", [IN_DIM, n_rows], F16, kind="ExternalInput")
    wad_t = nc.dram_tensor("wad_pd", [IN_DIM, H2], F16, kind="ExternalInput")
    out_t = nc.dram_tensor("statv", [H2, n_rows], F32, kind="ExternalOutput")
    with tile.TileContext(nc) as tc:
        with (tc.tile_pool(name="c", bufs=1) as cp,
              tc.tile_pool(name="s", bufs=3) as sp,
              tc.tile_pool(name="p2", bufs=3, space="PSUM") as pp2):
            wad_c = cp.tile([IN_DIM, H2], F16)
            nc.sync.dma_start(wad_c[:], wad_t.ap())
            for _rep in range(repeat):
                for n0 in range(0, n_rows, TW):
                    p = min(TW, n_rows - n0)
                    xt8 = sp.tile([128, TW], F16, tag="xt")
                    nc.sync.dma_start(xt8[:, :p], xs_t.ap()[:, n0:n0 + p])
                    av_ps = pp2.tile([H2, TW], F32, tag="av")
                    nc.tensor.matmul(av_ps[:, :p], lhsT=wad_c[:], rhs=xt8[:, :p],
                                     start=True, stop=True)
                    av8 = sp.tile([H2, TW], F32, tag="av8")
                    nc.vector.tensor_copy(av8[:, :p], av_ps[:, :p])
                    nc.sync.dma_start(out_t.ap()[:, n0:n0 + p], av8[:, :p])
    nc.compile()
    return nc


def preprocess(edge_index, n_all, npc, n_cores):
    """Static schedules: fixed 32-node groups, per-group block counts padded
    to the max over cores (one SPMD module). Pure numpy indexing."""
    src = np.asarray(edge_index[0], dtype=np.int64)
    dst = np.asarray(edge_index[1], dtype=np.int64)
    loop = np.arange(n_all, dtype=np.int64)
    src = np.concatenate([src, loop])
    dst = np.concatenate([dst, loop])
    order = np.argsort(dst, kind="stable")
    src = src[order]
    dst = dst[order]

    n_grp = (npc + WIN - 1) // WIN
    g_pad = ((n_grp + GPB - 1) // GPB) * GPB
    nsb = g_pad // GPB
    pad_n = g_pad * WIN - npc
    core_bounds = np.searchsorted(dst, np.arange(n_cores + 1) * npc)

    # stage A: per core, degree-balanced assignment of nodes to 32-node
    # groups (minimizes per-group block counts AND aligns them across
    # cores so the SPMD max-over-cores padding is tight), then per-group
    # lo/hi edge arrays. Device rows come out in group-slot order; run()
    # un-permutes on the host (indexing only).
    per_cg = []
    perms = []
    for c in range(n_cores):
        b0, b1 = core_bounds[c], core_bounds[c + 1]
        s = src[b0:b1]
        d = (dst[b0:b1] - c * npc).astype(np.int64)
        if pad_n:  # virtual degree-1 edges for pad slots
            s = np.concatenate([s, np.zeros(pad_n, dtype=np.int64)])
            d = np.concatenate([d, np.arange(npc, npc + pad_n, dtype=np.int64)])
        ntot = g_pad * WIN
        lo_m = s < SPLIT
        deg_lo = np.bincount(d[lo_m], minlength=ntot).astype(np.float64)
        deg_hi = np.bincount(d[~lo_m], minlength=ntot).astype(np.float64)
        G = g_pad
        cnt = np.zeros(G, np.int64)
        slo = np.zeros(G, np.int64)
        shi = np.zeros(G, np.int64)
        g_of = np.empty(ntot, np.int64)
        c_of = np.empty(ntot, np.int64)
        dl = deg_lo.astype(np.int64)
        dh = deg_hi.astype(np.int64)
        # greedy bin packing that directly minimizes block-count (ceil)
        # increments; groups end up filled to just under 128-multiples
        for n in np.argsort(-(dl + dh), kind="stable"):
            nlo, nhi = dl[n], dh[n]
            db = (((slo + nlo + BLK - 1) // BLK) - ((slo + BLK - 1) // BLK)
                  + ((shi + nhi + BLK - 1) // BLK) - ((shi + BLK - 1) // BLK))
            # secondary: prefer landing closest to a block boundary
            rem = ((-(slo + nlo)) % BLK) + ((-(shi + nhi)) % BLK)
            score = db * 1024 + (rem >> 3)
            score[cnt >= WIN] = 1 << 30
            g = int(np.argmin(score))
            g_of[n] = g
            c_of[n] = cnt[g]
            cnt[g] += 1
            slo[g] += nlo
            shi[g] += nhi
        # schedule slot k = k-th group by descending block needs (aligns
        # the per-slot maxima across cores)
        gorder = np.lexsort((-shi, -slo,
                             -((slo + BLK - 1) // BLK + (shi + BLK - 1) // BLK)))
        slot_of = np.empty(G, np.int64)
        slot_of[gorder] = np.arange(G)
        eg = slot_of[g_of[d]]                       # edge -> schedule slot
        order2 = np.lexsort((s, eg))                # slot-major, src-sorted
        s2, d2, eg2 = s[order2], d[order2], eg[order2]
        gb = np.searchsorted(eg2, np.arange(G + 1))
        rows = []
        for g in range(G):
            e0, e1 = gb[g], gb[g + 1]
            gs = s2[e0:e1]
            gc = c_of[d2[e0:e1]]
            gdst = np.minimum(c * npc + d2[e0:e1], n_all - 1)
            m = gs < SPLIT
            rows.append(((gs[m], gc[m], gdst[m]),
                         (gs[~m] - SPLIT, gc[~m], gdst[~m])))
        per_cg.append(rows)
        # perm[r]: device row r = slot k*WIN + c -> local node id (or -1)
        perm = np.full(ntot, -1, dtype=np.int64)
        node_rows = slot_of[g_of] * WIN + c_of      # node -> device row
        nodes = np.arange(ntot)
        perm[node_rows] = np.where(nodes < npc, nodes, -1)
        perms.append(perm)

    # stage B: global per-group block counts (max over cores)
    nlo_g = [max((len(per_cg[c][g][0][0]) + BLK - 1) // BLK
                 for c in range(n_cores)) for g in range(g_pad)]
    nhi_g = [max((len(per_cg[c][g][1][0]) + BLK - 1) // BLK
                 for c in range(n_cores)) for g in range(g_pad)]
    sched = []
    for sb in range(nsb):
        gs = range(sb * GPB, (sb + 1) * GPB)
        sched.append(([nlo_g[g] for g in gs], [nhi_g[g] for g in gs]))

    # stage C: per-core padded stream arrays
    def pad_block(vals, nblk, fill, dtype):
        a = np.full(nblk * BLK, fill, dtype=dtype)
        a[:len(vals)] = vals
        return a

    cores = []
    for c in range(n_cores):
        sbs = []
        for sb in range(nsb):
            gl = range(sb * GPB, (sb + 1) * GPB)
            idx_parts, col_parts, src_parts, dst_parts = [], [], [], []
            for half in (0, 1):
                cnt_g = nlo_g if half == 0 else nhi_g
                for g in gl:
                    hs, hc, hd = per_cg[c][g][half]
                    nb = cnt_g[g]
                    if nb == 0:
                        continue
                    idx_parts.append((half, pad_block(hs, nb, 0, np.int64)))
                    col_parts.append(pad_block(hc.astype(np.float16), nb,
                                               DEAD, np.float16))
                    src_parts.append(pad_block(
                        hs + (0 if half == 0 else SPLIT), nb, 0, np.int64))
                    dst_parts.append(pad_block(hd, nb, 0, np.int64))
            lo_idx = np.concatenate([a for h, a in idx_parts if h == 0]) \
                if any(h == 0 for h, _ in idx_parts) else np.zeros(0, np.int64)
            hi_idx = np.concatenate([a for h, a in idx_parts if h == 1]) \
                if any(h == 1 for h, _ in idx_parts) else np.zeros(0, np.int64)
            colidx = np.concatenate(col_parts).reshape(-1, BLK)   # [nbk,128]
            srcid = np.concatenate(src_parts).reshape(-1, BLK)
            dstid = np.concatenate(dst_parts).reshape(-1, BLK)
            sbs.append(dict(
                idx_lo=_wrap16(lo_idx) if len(lo_idx) else
                    np.zeros((128, 0), np.int16),
                idx_hi=_wrap16(hi_idx) if len(hi_idx) else
                    np.zeros((128, 0), np.int16),
                colidx=np.ascontiguousarray(colidx.T),            # [128,nbk]
                srcid=srcid, dstid=dstid))
        cores.append(sbs)
    return nsb, sched, cores, perms


def build_streams(cores, statv):
    """Per-edge a_src/a_dst expansion (indexing only) + packed stream blob."""
    asrcv, adstv = statv[:HEADS], statv[HEADS:]             # [4, n_all] f32
    outs = []
    for sbs in cores:
        blobs = []
        for sb in sbs:
            a_s = np.moveaxis(asrcv[:, sb["srcid"]], 0, -1)  # [nbk,128,4]
            a_d = np.moveaxis(adstv[:, sb["dstid"]], 0, -1)
            a8 = np.concatenate([a_s, a_d], axis=2)          # [nbk,128,8]
            a8 = np.ascontiguousarray(
                a8.transpose(1, 0, 2).astype(np.float16))    # [128,nbk,8]
            blobs.append(np.concatenate(
                [sb["idx_lo"], sb["idx_hi"], sb["colidx"].view(np.int16),
                 a8.reshape(128, -1).view(np.int16)], axis=1))
        outs.append(np.ascontiguousarray(np.concatenate(blobs, axis=1)))
    return outs


# -------------------------------------------------------------- device side

def build_nc2(cfg: Cfg, sched):
    nc = bacc.Bacc("TRN2", target_bir_lowering=False, debug=False,
                   num_devices=cfg.n_cores, num_swdge_queues=cfg.nq,
                   dynamic_dma_scratch_size=cfg.scratch)
    npc = cfg.npc
    HD = HEADS * OUT_DIM
    nsb = len(sched)
    nbk_s = [sum(l) + sum(h) for l, h in sched]
    TOT = sum(17 * b for b in nbk_s)
    nlo_max = max(sum(l) for l, _ in sched)
    nhi_max = max(sum(h) for _, h in sched)
    nbk_max = max(nbk_s)

    x_t = nc.dram_tensor("x16", [N_NODES, IN_DIM], F16, kind="ExternalInput")
    wf_t = nc.dram_tensor("wf", [IN_DIM, HD], F16, kind="ExternalInput")
    iota_t = nc.dram_tensor("iota16", [128, WIN], F16, kind="ExternalInput")
    biasb_t = nc.dram_tensor("biasb", [128, HD], F32, kind="ExternalInput")
    st_t = nc.dram_tensor("streams", [128, TOT], I16, kind="ExternalInput")
    npad = nsb * GPB * WIN
    out_t = nc.dram_tensor("out", [npad, HD], F32, kind="ExternalOutput")

    with tile.TileContext(nc) as tc:
        with tc.tile_pool(name="consts", bufs=1) as cpool:
            wf_c = cpool.tile([IN_DIM, HD], F16)
            nc.sync.dma_start(wf_c[:], wf_t.ap())
            iota_c = cpool.tile([128, WIN], F16)
            nc.sync.dma_start(iota_c[:], iota_t.ap())
            biasb_c = cpool.tile([128, HD], F32)
            nc.sync.dma_start(biasb_c[:], biasb_t.ap())
            ones_c = cpool.tile([128, 1], F16)
            nc.vector.memset(ones_c[:], 1.0)

            with (tc.tile_pool(name="pb_g", bufs=3) as gp,
                  tc.tile_pool(name="pb_m", bufs=2) as mp,
                  tc.tile_pool(name="pb_s", bufs=3) as sp,
                  tc.tile_pool(name="pb_z", bufs=2) as zp,
                  tc.tile_pool(name="pb_u", bufs=3, space="PSUM") as pu,
                  tc.tile_pool(name="pb_d", bufs=2, space="PSUM") as pdp,
                  tc.tile_pool(name="pb_w", bufs=2, space="PSUM") as pw):
                BL = cfg.blevel
                qi = 0
                for _rep in range(cfg.repeat):
                    off = 0
                    for sb in range(nsb):
                        nlo_l, nhi_l = sched[sb]
                        nlo, nhi = sum(nlo_l), sum(nhi_l)
                        nbk = nlo + nhi
                        W_sb = 17 * nbk
                        S0 = 8 * nlo
                        S1 = 8 * nbk
                        S2 = S1 + nbk
                        strm = sp.tile([128, 17 * nbk_max], I16, tag="strm")
                        nc.sync.dma_start(strm[:, :W_sb],
                                          st_t.ap()[:, off:off + W_sb])
                        off += W_sb
                        il = strm[:, 0:S0]
                        ih = strm[:, S0:S1]
                        cx = strm[:, S1:S2].bitcast(F16)
                        a8 = (strm[:, S2:W_sb].bitcast(F16)
                              .rearrange("p (b k) -> p b k", k=8))

                        glo = gp.tile([128, nlo_max, IN_DIM], F16, tag="glo")
                        for j0, nj in _gather_chunks(nlo * BLK, cfg.gnj):
                            nc.gpsimd.dma_gather(
                                glo[:, j0 // 128:(j0 + nj) // 128, :],
                                x_t.ap()[0:SPLIT, :],
                                il[:, j0 // 16:(j0 + nj) // 16],
                                nj, nj, IN_DIM, single_packet=cfg.sp,
                                queue_num=qi % cfg.nq)
                            qi += 1
                        ghi = gp.tile([128, nhi_max, IN_DIM], F16, tag="ghi")
                        for j0, nj in _gather_chunks(nhi * BLK, cfg.gnj):
                            nc.gpsimd.dma_gather(
                                ghi[:, j0 // 128:(j0 + nj) // 128, :],
                                x_t.ap()[SPLIT:N_NODES, :],
                                ih[:, j0 // 16:(j0 + nj) // 16],
                                nj, nj, IN_DIM, single_packet=cfg.sp,
                                queue_num=qi % cfg.nq)
                            qi += 1

                        if BL < 2:
                            continue
                        # expa = exp(lrelu(a_src + a_dst))
                        asum = sp.tile([128, nbk_max, HEADS], F32, tag="asum")
                        nc.vector.tensor_tensor(out=asum[:, :nbk, :],
                                                in0=a8[:, :, 0:HEADS],
                                                in1=a8[:, :, HEADS:8],
                                                op=mybir.AluOpType.add)
                        asc = sp.tile([128, nbk_max, HEADS], F32, tag="asc")
                        nc.vector.tensor_scalar(asc[:, :nbk, :],
                                                asum[:, :nbk, :], LEAKY_SLOPE,
                                                None, mybir.AluOpType.mult)
                        alr = sp.tile([128, nbk_max, HEADS], F32, tag="alr")
                        nc.vector.tensor_tensor(out=alr[:, :nbk, :],
                                                in0=asum[:, :nbk, :],
                                                in1=asc[:, :nbk, :],
                                                op=mybir.AluOpType.max)
                        expa = sp.tile([128, nbk_max, HEADS], F16, tag="expa")
                        nc.scalar.activation(expa[:, :nbk, :], alr[:, :nbk, :],
                                             mybir.ActivationFunctionType.Exp)
                        # onehot[e, b, c] = (iota[c] == colidx[e, b])
                        # Mw[e, b, h*32+c] = oneh * expa
                        # (built in two block-range chunks: head on DVE,
                        # tail on the otherwise-idle GPSIMD Q7 cores)
                        oneh = mp.tile([128, nbk_max, WIN], F16, tag="oneh")
                        mw = mp.tile([128, nbk_max, HEADS, WIN], F16, tag="mw")
                        nc.vector.tensor_tensor(
                            out=oneh[:, :nbk, :],
                            in0=iota_c[:].unsqueeze(1)
                                .to_broadcast([128, nbk, WIN]),
                            in1=cx.unsqueeze(2).to_broadcast([128, nbk, WIN]),
                            op=mybir.AluOpType.is_equal)
                        # mw multiply split: head on DVE, tail on the
                        # otherwise-idle GPSIMD (is_equal is DVE-only)
                        nsp = nbk - (nbk * cfg.poolpct) // 100
                        for b0, b1, eng in ((0, nsp, nc.vector),
                                            (nsp, nbk, nc.gpsimd)):
                            if b0 == b1:
                                continue
                            nb = b1 - b0
                            eng.tensor_tensor(
                                out=mw[:, b0:b1, :, :],
                                in0=oneh[:, b0:b1, :].unsqueeze(2)
                                    .to_broadcast([128, nb, HEADS, WIN]),
                                in1=expa[:, b0:b1, :].unsqueeze(3)
                                    .to_broadcast([128, nb, HEADS, WIN]),
                                op=mybir.AluOpType.mult)

                        if BL < 3:
                            continue
                        dt_ps = pdp.tile([128, GPB], F32, tag="dt")
                        zall_ps = pw.tile([WIN, GPB, HEADS, OUT_DIM], F32,
                                          tag="zall")
                        lo_c = np.cumsum([0] + nlo_l)
                        hi_c = np.cumsum([0] + nhi_l)
                        for g in range(GPB):
                            blocks = (
                                [(glo, lo_c[g] + j, lo_c[g] + j)
                                 for j in range(nlo_l[g])]
                                + [(ghi, hi_c[g] + j, nlo + hi_c[g] + j)
                                   for j in range(nhi_l[g])])
                            u2 = pu.tile([128, HD], F32, tag="u2")
                            for k, (gt, slot, bcol) in enumerate(blocks):
                                st = k == 0
                                sp_ = k == len(blocks) - 1
                                nc.tensor.matmul(u2[:],
                                                 lhsT=gt[:, slot, :],
                                                 rhs=mw[:, bcol, :, :],
                                                 start=st, stop=sp_)
                                nc.tensor.matmul(dt_ps[:, g:g + 1],
                                                 lhsT=mw[:, bcol, :, :],
                                                 rhs=ones_c[:],
                                                 start=st, stop=sp_)
                            if BL < 4:
                                continue
                            u2s = zp.tile([128, HD], F16, tag="u2s")
                            nc.scalar.activation(
                                u2s[:], u2[:],
                                mybir.ActivationFunctionType.Copy)
                            for h in range(HEADS):
                                nc.tensor.matmul(
                                    zall_ps[:, g, h, :],
                                    lhsT=u2s[:, h * WIN:(h + 1) * WIN],
                                    rhs=wf_c[:, h * OUT_DIM:(h + 1) * OUT_DIM],
                                    start=True, stop=True)
                        if BL < 4:
                            continue
                        # denominators -> [c, g, h]; normalize; swish; store
                        dts = zp.tile([128, GPB], F32, tag="dts")
                        nc.vector.tensor_copy(dts[:], dt_ps[:])
                        rec = zp.tile([WIN, GPB, HEADS], F32, tag="rec")
                        for h in range(HEADS):
                            nc.vector.tensor_copy(
                                rec[:, :, h], dts[h * WIN:(h + 1) * WIN, :])
                        nc.vector.reciprocal(rec[:], rec[:])
                        zn = zp.tile([WIN, GPB, HEADS, OUT_DIM], F32, tag="zn")
                        nc.vector.tensor_tensor(
                            out=zn[:], in0=zall_ps[:],
                            in1=rec[:].unsqueeze(3)
                                .to_broadcast([WIN, GPB, HEADS, OUT_DIM]),
                            op=mybir.AluOpType.mult)
                        if cfg.bias_nonzero:
                            nc.vector.tensor_tensor(
                                out=zn[:], in0=zn[:],
                                in1=biasb_c[0:WIN, :]
                                    .rearrange("c (h o) -> c h o", o=OUT_DIM)
                                    .unsqueeze(1)
                                    .to_broadcast([WIN, GPB, HEADS, OUT_DIM]),
                                op=mybir.AluOpType.add)
                        sg = zp.tile([WIN, GPB, HEADS, OUT_DIM], F16, tag="sg")
                        nc.scalar.activation(
                            sg[:], zn[:], mybir.ActivationFunctionType.Sigmoid)
                        mix = zp.tile([WIN, GPB, HEADS, OUT_DIM], F16,
                                      tag="mix")
                        nc.vector.tensor_scalar(mix[:], sg[:], CMIX - BETA,
                                                BETA, mybir.AluOpType.mult,
                                                mybir.AluOpType.add)
                        zrow = zp.tile([WIN, GPB, HEADS, OUT_DIM], F32,
                                       tag="zrow")
                        nc.vector.tensor_tensor(out=zrow[:], in0=zn[:],
                                                in1=mix[:],
                                                op=mybir.AluOpType.mult)
                        n0 = sb * GPB * WIN
                        nc.sync.dma_start(
                            out_t.ap()[n0:n0 + GPB * WIN, :]
                                 .rearrange("(g c) (h o) -> c g h o",
                                            c=WIN, o=OUT_DIM),
                            zrow[:])
    nc.compile()
    return nc


# ---------------------------------------------------------------- the API

def run(x, edge_index, W, att_src, att_dst, bias, trace=False, cfg_kw=None):
    npc = N_NODES // N_CORES
    nsb, sched, cores, perms = preprocess(edge_index, N_NODES, npc, N_CORES)

    x16 = np.asarray(x, np.float32).astype(np.float16)      # [N, 128]
    W32 = np.asarray(W, dtype=np.float32)
    as32 = np.asarray(att_src, dtype=np.float32)
    ad32 = np.asarray(att_dst, dtype=np.float32)
    bias32 = np.asarray(bias, dtype=np.float32)
    S = np.zeros((HEADS * OUT_DIM, 2 * HEADS), dtype=np.float32)
    for h in range(HEADS):
        S[h * OUT_DIM:(h + 1) * OUT_DIM, h] = as32[h]
        S[h * OUT_DIM:(h + 1) * OUT_DIM, HEADS + h] = ad32[h]
    wad = (W32 @ S).astype(np.float16)          # param-only host matmul

    # launch 1: per-node attention stats
    nc1 = build_nc_stats(npc, N_CORES)
    in_maps1 = [dict(
        xT_slab=np.ascontiguousarray(x16[c * npc:(c + 1) * npc].T),
        wad_pd=wad) for c in range(N_CORES)]
    res1 = run_bass_kernel_spmd(nc1, in_maps1, core_ids=list(range(N_CORES)),
                                trace=trace)
    statv = np.concatenate(
        [res1.results[c]["statv"] for c in range(N_CORES)], axis=1)

    streams = build_streams(cores, statv)
    bias_nonzero = bool(np.any(bias32))
    kw = dict(cfg_kw or {})
    cfg = Cfg(npc, n_cores=N_CORES, bias_nonzero=bias_nonzero, **kw)

    nc2 = build_nc2(cfg, sched)
    iota = np.tile(np.arange(WIN, dtype=np.float16), (128, 1))
    biasb = np.tile(bias32, (128, 1)).astype(np.float32)
    wf16 = W32.astype(np.float16)
    in_maps = [dict(x16=x16, wf=wf16, iota16=iota, biasb=biasb,
                    streams=streams[c]) for c in range(N_CORES)]
    res = run_bass_kernel_spmd(nc2, in_maps, core_ids=list(range(N_CORES)),
                               trace=trace)
    out = np.empty((N_NODES, HEADS * OUT_DIM), np.float32)
    for c in range(N_CORES):
        dev = res.results[c]["out"]                 # [npad, HD] slot order
        perm = perms[c]
        valid = perm >= 0
        out[c * npc + perm[valid]] = dev[valid]
    parts = dict(nc1=nc1, in_maps1=in_maps1, nc2=nc2, in_maps2=in_maps,
                 res1=res1, res2=res, n_cores=N_CORES, cfg=cfg, npc=npc,
                 sched=sched)
    return out, parts


def make_pjrt_fn(nc, in_maps, n_cores):
    """Build a jitted PJRT executor for a prebuilt Bass module (axon path).
    Returns (fn, args); inputs are pre-staged on device."""
    import jax
    from jax.sharding import Mesh, NamedSharding, PartitionSpec
    from jax.experimental.shard_map import shard_map

    import concourse.mybir as mybir_
    from concourse import bass2jax as b2j

    b2j.install_neuronx_cc_hook()
    partition_name = (nc.partition_id_tensor.name
                      if nc.partition_id_tensor else None)
    in_names, out_names, out_avals, zero_outs = [], [], [], []
    for alloc in nc.m.functions[0].allocations:
        if not isinstance(alloc, mybir_.MemoryLocationSet):
            continue
        name = alloc.memorylocations[0].name
        if alloc.kind == "ExternalInput":
            if name != partition_name:
                in_names.append(name)
        elif alloc.kind == "ExternalOutput":
            dt = mybir_.dt.np(alloc.dtype)
            out_avals.append(jax.core.ShapedArray(tuple(alloc.tensor_shape), dt))
            out_names.append(name)
            zero_outs.append(np.zeros(tuple(alloc.tensor_shape), dt))

    # the bind's in_names must cover ALL operands (inputs + zero-out bufs
    # + partition id) — neuronx_cc_hook asserts len(in_names) == n_operands.
    bind_names = list(in_names) + list(out_names)
    if partition_name is not None:
        bind_names.append(partition_name)

    def _body(*args):
        operands = list(args)
        if partition_name is not None:
            operands.append(b2j.partition_id_tensor())
        outs = b2j._bass_exec_p.bind(
            *operands, out_avals=tuple(out_avals), in_names=tuple(bind_names),
            out_names=tuple(out_names), lowering_input_output_aliases=(),
            sim_require_finite=True, sim_require_nnan=True, nc=nc)
        return tuple(outs)

    n_params = len(in_names)
    devices = jax.devices()[:n_cores]
    mesh = Mesh(np.asarray(devices), ("core",))
    spec = PartitionSpec("core")
    fn = jax.jit(shard_map(_body, mesh=mesh,
                           in_specs=(spec,) * (n_params + len(zero_outs)),
                           out_specs=(spec,) * len(out_names),
                           check_rep=False), keep_unused=True)
    sh = NamedSharding(mesh, spec)
    args = [jax.device_put(
                np.concatenate([in_maps[c][nm] for c in range(n_cores)], 0), sh)
            for nm in in_names]
    args += [jax.device_put(
                np.zeros((n_cores * z.shape[0], *z.shape[1:]), z.dtype), sh)
             for z in zero_outs]
    return fn, args


def bench_pair(fnA, argsA, fnB, argsB, iters=24):
    """Interleaved wall-clock of two executables with a 4-byte D2H fetch as
    the completion sync (block_until_ready alone is lost in ~40-90ms axon RPC
    noise; interleaving + cluster-min cancels the shared offset)."""
    import time as _time
    for fn, args in ((fnA, argsA), (fnB, argsB)):
        r = fn(*args)
        _ = np.asarray(r[0][0:1, 0:1])
    pa, pb = [], []
    for _ in range(iters):
        t0 = _time.perf_counter()
        r = fnA(*argsA)
        _ = np.asarray(r[0][0:1, 0:1])
        pa.append(_time.perf_counter() - t0)
        t0 = _time.perf_counter()
        r = fnB(*argsB)
        _ = np.asarray(r[0][0:1, 0:1])
        pb.append(_time.perf_counter() - t0)
    return np.array(pa), np.array(pb)


def bench_slope(ncA, ncB, in_maps, n_cores, reps, iters=24):
    """HW ns of one kernel body via the repeat-slope method: ncA has repeat=1,
    ncB has repeat=reps; returns (per-rep seconds, raw pair arrays)."""
    fnA, argsA = make_pjrt_fn(ncA, in_maps, n_cores)
    fnB, argsB = make_pjrt_fn(ncB, in_maps, n_cores)
    pa, pb = bench_pair(fnA, argsA, fnB, argsB, iters=iters)
    # medians: robust against the occasional anomalous fast RPC round-trip
    per = (np.median(pb) - np.median(pa)) / (reps - 1)
    return max(0.0, per), (pa, pb)


def kernel(**inputs) -> np.ndarray:
    out, _ = run(inputs["x"], inputs["edge_index"], inputs["W"],
                 inputs["att_src"], inputs["att_dst"], inputs["bias"])
    return out


# revision 18
# speedup vs baseline: 2.1603x; 2.1603x over previous
# MixGAT layer (GATConv + beta-mix swish) on 8 Trainium2 NeuronCores, v2.
#
# Strategy (dst-node sharding):
#  - Nodes partitioned across 8 cores by dst id; each core owns N/8 dst rows.
#  - KEY CHANGE vs v1: aggregation is linear in xp = x @ W, so we aggregate
#    RAW x features per dst and project ONCE per dst node afterwards:
#      out[d] = (softmax-weighted-sum_e x[src_e]) / denom @ W
#    This removes the on-device projection-table build (old phase A): the
#    gather table is just x cast to f16 on the host (node-major, 256B rows
#    instead of 512B).
#  - Launch 1 (tiny): per-node attention stats a_src/a_dst = x @ (W@att) on
#    device; host expands them into per-edge streams (indexing only).
#  - Launch 2, per superblock (128 dst nodes = 4 fixed 32-node groups):
#      dma_gather x rows per edge slot (lo/hi int16-index split, 256B rows,
#      single-packet descriptors spread over 4 SWDGE queues),
#      expa = exp(lrelu(a_src+a_dst)) from streams,
#      Mw[e, h*32+c] = expa[e, h] * onehot(dst slot c),
#      per 128-edge block:  U2[feat, slot] += glo_blk(lhsT) @ Mw(rhs)
#                           Dt[slot, 1]   += Mw(lhsT) @ ones(rhs)
#      per group: project  Z[c, h*32+o] = U2[:, h*32..](lhsT) @ W[:, h*32..]
#      per sb: denominators to [c, g, h] via 4 partition-shifted copies,
#      normalize, beta-mix swish, ONE node-ordered output DMA.
#    Fixed 32-node groups keep outputs contiguous: no scratch roundtrip and
#    no permutation pass. Per-group block counts are padded to the max over
#    cores so one SPMD module serves all 8 cores.
#
# kernel(**inputs) is self-contained: preprocessing is pure numpy (sorting /
# indexing / dtype casts only), device kernels built with bass/Tile, run via
# run_bass_kernel_spmd on cores 0-7.

import numpy as np

import concourse.bass as bass
import concourse.mybir as mybir
import concourse.tile as tile
from concourse import bacc
from concourse.bass_utils import run_bass_kernel_spmd

F32 = mybir.dt.float32
F16 = mybir.dt.float16
I16 = mybir.dt.int16

# problem constants
N_NODES = 50000
IN_DIM = 128
HEADS = 4
OUT_DIM = 32
LEAKY_SLOPE = 0.2
BETA = 0.5
CMIX = 1.2
N_CORES = 8

# static schedule constants
WIN = 32          # dst nodes per group (PSUM slots = HEADS*WIN = 128)
BLK = 128         # edges per block (gather slots -> partitions)
GPB = 4           # groups per superblock (4*32 = 128 dst nodes)
SPLIT = 32768     # int16-addressable table split
DEAD = 100.0      # colidx value for dead slots (never equals iota 0..31)
GNJ = 1024        # rows per dma_gather call (SWDGE ring holds scratch/16
                  # descriptors; stay strictly under that at 1 desc/row)
NQ = 4            # SWDGE queues to spread gathers over
SCRATCH = 32768   # dynamic dma scratch (ring) bytes per partition
SINGLE_PACKET = True


def _wrap16(v):
    """idx vector [S*16] -> dma_gather idx layout [128, S]."""
    s = v.reshape(-1, 16).T                      # [16, S]
    return np.tile(s, (8, 1)).astype(np.int16)   # [128, S]


def _gather_chunks(total, gnj):
    out = []
    o = 0
    while o < total:
        c = min(gnj, total - o)
        out.append((o, c))
        o += c
    return out


class Cfg:
    def __init__(self, npc, n_cores=N_CORES, bias_nonzero=False, repeat=1,
                 blevel=4, gnj=GNJ, nq=NQ, scratch=SCRATCH, sp=SINGLE_PACKET,
                 poolpct=0):
        self.npc = npc
        self.n_cores = n_cores
        self.bias_nonzero = bias_nonzero
        self.repeat = repeat
        self.blevel = blevel   # 1 gather only; 2 +mw; 3 +matmul; 4 full
        self.gnj = gnj
        self.nq = nq
        self.scratch = scratch
        self.sp = sp
        self.poolpct = poolpct  # % of oneh/mw blocks built on GPSIMD


# ---------------------------------------------------------------- host side

def build_nc_stats(n_rows, n_cores, repeat=1):
    """Launch-1 mini kernel: statv[8, n_rows] = (W@[as|ad]).T @ xT_slab."""
    nc = bacc.Bacc("TRN2", target_bir_lowering=False, debug=False,
                   num_devices=n_cores)
    TW = 512
    H2 = 2 * HEADS
    xs_t = nc.dram_tensor("xT_slab", [IN_DIM, n_rows], F16, kind="ExternalInput")
    wad_t = nc.dram_tensor("wad_pd", [IN_DIM, H2], F16, kind="ExternalInput")
    out_t = nc.dram_tensor("statv", [H2, n_rows], F32, kind="ExternalOutput")
    with tile.TileContext(nc) as tc:
        with (tc.tile_pool(name="c", bufs=1) as cp,
              tc.tile_pool(name="s", bufs=3) as sp,
              tc.tile_pool(name="p2", bufs=3, space="PSUM") as pp2):
            wad_c = cp.tile([IN_DIM, H2], F16)
            nc.sync.dma_start(wad_c[:], wad_t.ap())
            for _rep in range(repeat):
                for n0 in range(0, n_rows, TW):
                    p = min(TW, n_rows - n0)
                    xt8 = sp.tile([128, TW], F16, tag="xt")
                    nc.sync.dma_start(xt8[:, :p], xs_t.ap()[:, n0:n0 + p])
                    av_ps = pp2.tile([H2, TW], F32, tag="av")
                    nc.tensor.matmul(av_ps[:, :p], lhsT=wad_c[:], rhs=xt8[:, :p],
                                     start=True, stop=True)
                    av8 = sp.tile([H2, TW], F32, tag="av8")
                    nc.vector.tensor_copy(av8[:, :p], av_ps[:, :p])
                    nc.sync.dma_start(out_t.ap()[:, n0:n0 + p], av8[:, :p])
    nc.compile()
    return nc


def preprocess(edge_index, n_all, npc, n_cores):
    """Static schedules: fixed 32-node groups, per-group block counts padded
    to the max over cores (one SPMD module). Pure numpy indexing."""
    src = np.asarray(edge_index[0], dtype=np.int64)
    dst = np.asarray(edge_index[1], dtype=np.int64)
    loop = np.arange(n_all, dtype=np.int64)
    src = np.concatenate([src, loop])
    dst = np.concatenate([dst, loop])
    order = np.argsort(dst, kind="stable")
    src = src[order]
    dst = dst[order]

    n_grp = (npc + WIN - 1) // WIN
    g_pad = ((n_grp + GPB - 1) // GPB) * GPB
    nsb = g_pad // GPB
    pad_n = g_pad * WIN - npc
    core_bounds = np.searchsorted(dst, np.arange(n_cores + 1) * npc)

    # stage A: per core, degree-balanced assignment of nodes to 32-node
    # groups (minimizes per-group block counts AND aligns them across
    # cores so the SPMD max-over-cores padding is tight), then per-group
    # lo/hi edge arrays. Device rows come out in group-slot order; run()
    # un-permutes on the host (indexing only).
    per_cg = []
    perms = []
    for c in range(n_cores):
        b0, b1 = core_bounds[c], core_bounds[c + 1]
        s = src[b0:b1]
        d = (dst[b0:b1] - c * npc).astype(np.int64)
        if pad_n:  # virtual degree-1 edges for pad slots
            s = np.concatenate([s, np.zeros(pad_n, dtype=np.int64)])
            d = np.concatenate([d, np.arange(npc, npc + pad_n, dtype=np.int64)])
        ntot = g_pad * WIN
        lo_m = s < SPLIT
        deg_lo = np.bincount(d[lo_m], minlength=ntot).astype(np.float64)
        deg_hi = np.bincount(d[~lo_m], minlength=ntot).astype(np.float64)
        G = g_pad
        cnt = np.zeros(G, np.int64)
        slo = np.zeros(G, np.int64)
        shi = np.zeros(G, np.int64)
        g_of = np.empty(ntot, np.int64)
        c_of = np.empty(ntot, np.int64)
        dl = deg_lo.astype(np.int64)
        dh = deg_hi.astype(np.int64)
        # greedy bin packing that directly minimizes block-count (ceil)
        # increments; groups end up filled to just under 128-multiples
        for n in np.argsort(-(dl + dh), kind="stable"):
            nlo, nhi = dl[n], dh[n]
            db = (((slo + nlo + BLK - 1) // BLK) - ((slo + BLK - 1) // BLK)
                  + ((shi + nhi + BLK - 1) // BLK) - ((shi + BLK - 1) // BLK))
            # secondary: prefer landing closest to a block boundary
            rem = ((-(slo + nlo)) % BLK) + ((-(shi + nhi)) % BLK)
            score = db * 1024 + (rem >> 3)
            score[cnt >= WIN] = 1 << 30
            g = int(np.argmin(score))
            g_of[n] = g
            c_of[n] = cnt[g]
            cnt[g] += 1
            slo[g] += nlo
            shi[g] += nhi
        # schedule slot k = k-th group by descending block needs (aligns
        # the per-slot maxima across cores)
        gorder = np.lexsort((-shi, -slo,
                             -((slo + BLK - 1) // BLK + (shi + BLK - 1) // BLK)))
        slot_of = np.empty(G, np.int64)
        slot_of[gorder] = np.arange(G)
        eg = slot_of[g_of[d]]                       # edge -> schedule slot
        order2 = np.lexsort((s, eg))                # slot-major, src-sorted
        s2, d2, eg2 = s[order2], d[order2], eg[order2]
        gb = np.searchsorted(eg2, np.arange(G + 1))
        rows = []
        for g in range(G):
            e0, e1 = gb[g], gb[g + 1]
            gs = s2[e0:e1]
            gc = c_of[d2[e0:e1]]
            gdst = np.minimum(c * npc + d2[e0:e1], n_all - 1)
            m = gs < SPLIT
            rows.append(((gs[m], gc[m], gdst[m]),
                         (gs[~m] - SPLIT, gc[~m], gdst[~m])))
        per_cg.append(rows)
        # perm[r]: device row r = slot k*WIN + c -> local node id (or -1)
        perm = np.full(ntot, -1, dtype=np.int64)
        node_rows = slot_of[g_of] * WIN + c_of      # node -> device row
        nodes = np.arange(ntot)
        perm[node_rows] = np.where(nodes < npc, nodes, -1)
        perms.append(perm)

    # stage B: global per-group block counts (max over cores)
    nlo_g = [max((len(per_cg[c][g][0][0]) + BLK - 1) // BLK
                 for c in range(n_cores)) for g in range(g_pad)]
    nhi_g = [max((len(per_cg[c][g][1][0]) + BLK - 1) // BLK
                 for c in range(n_cores)) for g in range(g_pad)]
    sched = []
    for sb in range(nsb):
        gs = range(sb * GPB, (sb + 1) * GPB)
        sched.append(([nlo_g[g] for g in gs], [nhi_g[g] for g in gs]))

    # stage C: per-core padded stream arrays
    def pad_block(vals, nblk, fill, dtype):
        a = np.full(nblk * BLK, fill, dtype=dtype)
        a[:len(vals)] = vals
        return a

    cores = []
    for c in range(n_cores):
        sbs = []
        for sb in range(nsb):
            gl = range(sb * GPB, (sb + 1) * GPB)
            idx_parts, col_parts, src_parts, dst_parts = [], [], [], []
            for half in (0, 1):
                cnt_g = nlo_g if half == 0 else nhi_g
                for g in gl:
                    hs, hc, hd = per_cg[c][g][half]
                    nb = cnt_g[g]
                    if nb == 0:
                        continue
                    idx_parts.append((half, pad_block(hs, nb, 0, np.int64)))
                    col_parts.append(pad_block(hc.astype(np.float16), nb,
                                               DEAD, np.float16))
                    src_parts.append(pad_block(
                        hs + (0 if half == 0 else SPLIT), nb, 0, np.int64))
                    dst_parts.append(pad_block(hd, nb, 0, np.int64))
            lo_idx = np.concatenate([a for h, a in idx_parts if h == 0]) \
                if any(h == 0 for h, _ in idx_parts) else np.zeros(0, np.int64)
            hi_idx = np.concatenate([a for h, a in idx_parts if h == 1]) \
                if any(h == 1 for h, _ in idx_parts) else np.zeros(0, np.int64)
            colidx = np.concatenate(col_parts).reshape(-1, BLK)   # [nbk,128]
            srcid = np.concatenate(src_parts).reshape(-1, BLK)
            dstid = np.concatenate(dst_parts).reshape(-1, BLK)
            sbs.append(dict(
                idx_lo=_wrap16(lo_idx) if len(lo_idx) else
                    np.zeros((128, 0), np.int16),
                idx_hi=_wrap16(hi_idx) if len(hi_idx) else
                    np.zeros((128, 0), np.int16),
                colidx=np.ascontiguousarray(colidx.T),            # [128,nbk]
                srcid=srcid, dstid=dstid))
        cores.append(sbs)
    return nsb, sched, cores, perms


def build_streams(cores, statv):
    """Per-edge a_src/a_dst expansion (indexing only) + packed stream blob."""
    asrcv, adstv = statv[:HEADS], statv[HEADS:]             # [4, n_all] f32
    outs = []
    for sbs in cores:
        blobs = []
        for sb in sbs:
            a_s = np.moveaxis(asrcv[:, sb["srcid"]], 0, -1)  # [nbk,128,4]
            a_d = np.moveaxis(adstv[:, sb["dstid"]], 0, -1)
            a8 = np.concatenate([a_s, a_d], axis=2)          # [nbk,128,8]
            a8 = np.ascontiguousarray(
                a8.transpose(1, 0, 2).astype(np.float16))    # [128,nbk,8]
            blobs.append(np.concatenate(
                [sb["idx_lo"], sb["idx_hi"], sb["colidx"].view(np.int16),
                 a8.reshape(128, -1).view(np.int16)], axis=1))
        outs.append(np.ascontiguousarray(np.concatenate(blobs, axis=1)))
    return outs


# -------------------------------------------------------------- device side

def build_nc2(cfg: Cfg, sched):
    nc = bacc.Bacc("TRN2", target_bir_lowering=False, debug=False,
                   num_devices=cfg.n_cores, num_swdge_queues=cfg.nq,
                   dynamic_dma_scratch_size=cfg.scratch)
    npc = cfg.npc
    HD = HEADS * OUT_DIM
    nsb = len(sched)
    nbk_s = [sum(l) + sum(h) for l, h in sched]
    TOT = sum(17 * b for b in nbk_s)
    nlo_max = max(sum(l) for l, _ in sched)
    nhi_max = max(sum(h) for _, h in sched)
    nbk_max = max(nbk_s)

    x_t = nc.dram_tensor("x16", [N_NODES, IN_DIM], F16, kind="ExternalInput")
    wf_t = nc.dram_tensor("wf", [IN_DIM, HD], F16, kind="ExternalInput")
    iota_t = nc.dram_tensor("iota16", [128, WIN], F16, kind="ExternalInput")
    biasb_t = nc.dram_tensor("biasb", [128, HD], F32, kind="ExternalInput")
    st_t = nc.dram_tensor("streams", [128, TOT], I16, kind="ExternalInput")
    npad = nsb * GPB * WIN
    out_t = nc.dram_tensor("out", [npad, HD], F32, kind="ExternalOutput")

    with tile.TileContext(nc) as tc:
        with tc.tile_pool(name="consts", bufs=1) as cpool:
            wf_c = cpool.tile([IN_DIM, HD], F16)
            nc.sync.dma_start(wf_c[:], wf_t.ap())
            iota_c = cpool.tile([128, WIN], F16)
            nc.sync.dma_start(iota_c[:], iota_t.ap())
            biasb_c = cpool.tile([128, HD], F32)
            nc.sync.dma_start(biasb_c[:], biasb_t.ap())
            ones_c = cpool.tile([128, 1], F16)
            nc.vector.memset(ones_c[:], 1.0)

            with (tc.tile_pool(name="pb_g", bufs=3) as gp,
                  tc.tile_pool(name="pb_m", bufs=3) as mp,
                  tc.tile_pool(name="pb_s", bufs=3) as sp,
                  tc.tile_pool(name="pb_z", bufs=2) as zp,
                  tc.tile_pool(name="pb_u", bufs=3, space="PSUM") as pu,
                  tc.tile_pool(name="pb_d", bufs=2, space="PSUM") as pdp,
                  tc.tile_pool(name="pb_w", bufs=3, space="PSUM") as pw):
                BL = cfg.blevel
                qi = 0
                for _rep in range(cfg.repeat):
                    off = 0
                    for sb in range(nsb):
                        nlo_l, nhi_l = sched[sb]
                        nlo, nhi = sum(nlo_l), sum(nhi_l)
                        nbk = nlo + nhi
                        W_sb = 17 * nbk
                        S0 = 8 * nlo
                        S1 = 8 * nbk
                        S2 = S1 + nbk
                        strm = sp.tile([128, 17 * nbk_max], I16, tag="strm")
                        nc.sync.dma_start(strm[:, :W_sb],
                                          st_t.ap()[:, off:off + W_sb])
                        off += W_sb
                        il = strm[:, 0:S0]
                        ih = strm[:, S0:S1]
                        cx = strm[:, S1:S2].bitcast(F16)
                        a8 = (strm[:, S2:W_sb].bitcast(F16)
                              .rearrange("p (b k) -> p b k", k=8))

                        glo = gp.tile([128, nlo_max, IN_DIM], F16, tag="glo")
                        for j0, nj in _gather_chunks(nlo * BLK, cfg.gnj):
                            nc.gpsimd.dma_gather(
                                glo[:, j0 // 128:(j0 + nj) // 128, :],
                                x_t.ap()[0:SPLIT, :],
                                il[:, j0 // 16:(j0 + nj) // 16],
                                nj, nj, IN_DIM, single_packet=cfg.sp,
                                queue_num=qi % cfg.nq)
                            qi += 1
                        ghi = gp.tile([128, nhi_max, IN_DIM], F16, tag="ghi")
                        for j0, nj in _gather_chunks(nhi * BLK, cfg.gnj):
                            nc.gpsimd.dma_gather(
                                ghi[:, j0 // 128:(j0 + nj) // 128, :],
                                x_t.ap()[SPLIT:N_NODES, :],
                                ih[:, j0 // 16:(j0 + nj) // 16],
                                nj, nj, IN_DIM, single_packet=cfg.sp,
                                queue_num=qi % cfg.nq)
                            qi += 1

                        if BL < 2:
                            continue
                        # expa = exp(lrelu(a_src + a_dst)), all f16
                        asum = sp.tile([128, nbk_max, HEADS], F16, tag="asum")
                        nc.vector.tensor_tensor(out=asum[:, :nbk, :],
                                                in0=a8[:, :, 0:HEADS],
                                                in1=a8[:, :, HEADS:8],
                                                op=mybir.AluOpType.add)
                        asc = sp.tile([128, nbk_max, HEADS], F16, tag="asc")
                        nc.vector.tensor_scalar(asc[:, :nbk, :],
                                                asum[:, :nbk, :], LEAKY_SLOPE,
                                                None, mybir.AluOpType.mult)
                        alr = sp.tile([128, nbk_max, HEADS], F16, tag="alr")
                        nc.vector.tensor_tensor(out=alr[:, :nbk, :],
                                                in0=asum[:, :nbk, :],
                                                in1=asc[:, :nbk, :],
                                                op=mybir.AluOpType.max)
                        expa = sp.tile([128, nbk_max, HEADS], F16, tag="expa")
                        nc.scalar.activation(expa[:, :nbk, :], alr[:, :nbk, :],
                                             mybir.ActivationFunctionType.Exp)
                        # onehot[e, b, c] = (iota[c] == colidx[e, b])
                        # Mw[e, b, h*32+c] = oneh * expa
                        # (built in two block-range chunks: head on DVE,
                        # tail on the otherwise-idle GPSIMD Q7 cores)
                        oneh = mp.tile([128, nbk_max, WIN], F16, tag="oneh")
                        mw = mp.tile([128, nbk_max, HEADS, WIN], F16, tag="mw")
                        nc.vector.tensor_tensor(
                            out=oneh[:, :nbk, :],
                            in0=iota_c[:].unsqueeze(1)
                                .to_broadcast([128, nbk, WIN]),
                            in1=cx.unsqueeze(2).to_broadcast([128, nbk, WIN]),
                            op=mybir.AluOpType.is_equal)
                        # mw multiply split: head on DVE, tail on the
                        # otherwise-idle GPSIMD (is_equal is DVE-only)
                        nsp = nbk - (nbk * cfg.poolpct) // 100
                        for b0, b1, eng in ((0, nsp, nc.vector),
                                            (nsp, nbk, nc.gpsimd)):
                            if b0 == b1:
                                continue
                            nb = b1 - b0
                            eng.tensor_tensor(
                                out=mw[:, b0:b1, :, :],
                                in0=oneh[:, b0:b1, :].unsqueeze(2)
                                    .to_broadcast([128, nb, HEADS, WIN]),
                                in1=expa[:, b0:b1, :].unsqueeze(3)
                                    .to_broadcast([128, nb, HEADS, WIN]),
                                op=mybir.AluOpType.mult)

                        if BL < 3:
                            continue
                        dt_ps = pdp.tile([128, GPB], F32, tag="dt")
                        zall_ps = pw.tile([WIN, GPB, HEADS, OUT_DIM], F32,
                                          tag="zall")
                        lo_c = np.cumsum([0] + nlo_l)
                        hi_c = np.cumsum([0] + nhi_l)
                        for g in range(GPB):
                            blocks = (
                                [(glo, lo_c[g] + j, lo_c[g] + j)
                                 for j in range(nlo_l[g])]
                                + [(ghi, hi_c[g] + j, nlo + hi_c[g] + j)
                                   for j in range(nhi_l[g])])
                            u2 = pu.tile([128, HD], F32, tag="u2")
                            for k, (gt, slot, bcol) in enumerate(blocks):
                                st = k == 0
                                sp_ = k == len(blocks) - 1
                                nc.tensor.matmul(u2[:],
                                                 lhsT=gt[:, slot, :],
                                                 rhs=mw[:, bcol, :, :],
                                                 start=st, stop=sp_)
                                nc.tensor.matmul(dt_ps[:, g:g + 1],
                                                 lhsT=mw[:, bcol, :, :],
                                                 rhs=ones_c[:],
                                                 start=st, stop=sp_)
                            if BL < 4:
                                continue
                            u2s = zp.tile([128, HD], F16, tag="u2s")
                            nc.scalar.activation(
                                u2s[:], u2[:],
                                mybir.ActivationFunctionType.Copy)
                            for h in range(HEADS):
                                nc.tensor.matmul(
                                    zall_ps[:, g, h, :],
                                    lhsT=u2s[:, h * WIN:(h + 1) * WIN],
                                    rhs=wf_c[:, h * OUT_DIM:(h + 1) * OUT_DIM],
                                    start=True, stop=True)
                        if BL < 4:
                            continue
                        # denominators -> [c, g, h]; normalize; swish; store
                        dts = zp.tile([128, GPB], F32, tag="dts")
                        nc.vector.tensor_copy(dts[:], dt_ps[:])
                        rec = zp.tile([WIN, GPB, HEADS], F32, tag="rec")
                        for h in range(HEADS):
                            nc.vector.tensor_copy(
                                rec[:, :, h], dts[h * WIN:(h + 1) * WIN, :])
                        nc.vector.reciprocal(rec[:], rec[:])
                        zn = zp.tile([WIN, GPB, HEADS, OUT_DIM], F32, tag="zn")
                        nc.vector.tensor_tensor(
                            out=zn[:], in0=zall_ps[:],
                            in1=rec[:].unsqueeze(3)
                                .to_broadcast([WIN, GPB, HEADS, OUT_DIM]),
                            op=mybir.AluOpType.mult)
                        if cfg.bias_nonzero:
                            nc.vector.tensor_tensor(
                                out=zn[:], in0=zn[:],
                                in1=biasb_c[0:WIN, :]
                                    .rearrange("c (h o) -> c h o", o=OUT_DIM)
                                    .unsqueeze(1)
                                    .to_broadcast([WIN, GPB, HEADS, OUT_DIM]),
                                op=mybir.AluOpType.add)
                        sg = zp.tile([WIN, GPB, HEADS, OUT_DIM], F16, tag="sg")
                        nc.scalar.activation(
                            sg[:], zn[:], mybir.ActivationFunctionType.Sigmoid)
                        mix = zp.tile([WIN, GPB, HEADS, OUT_DIM], F16,
                                      tag="mix")
                        nc.vector.tensor_scalar(mix[:], sg[:], CMIX - BETA,
                                                BETA, mybir.AluOpType.mult,
                                                mybir.AluOpType.add)
                        zrow = zp.tile([WIN, GPB, HEADS, OUT_DIM], F32,
                                       tag="zrow")
                        nc.vector.tensor_tensor(out=zrow[:], in0=zn[:],
                                                in1=mix[:],
                                                op=mybir.AluOpType.mult)
                        n0 = sb * GPB * WIN
                        nc.sync.dma_start(
                            out_t.ap()[n0:n0 + GPB * WIN, :]
                                 .rearrange("(g c) (h o) -> c g h o",
                                            c=WIN, o=OUT_DIM),
                            zrow[:])
    nc.compile()
    return nc


# ---------------------------------------------------------------- the API

def run(x, edge_index, W, att_src, att_dst, bias, trace=False, cfg_kw=None):
    npc = N_NODES // N_CORES
    nsb, sched, cores, perms = preprocess(edge_index, N_NODES, npc, N_CORES)

    x16 = np.asarray(x, np.float32).astype(np.float16)      # [N, 128]
    W32 = np.asarray(W, dtype=np.float32)
    as32 = np.asarray(att_src, dtype=np.float32)
    ad32 = np.asarray(att_dst, dtype=np.float32)
    bias32 = np.asarray(bias, dtype=np.float32)
    S = np.zeros((HEADS * OUT_DIM, 2 * HEADS), dtype=np.float32)
    for h in range(HEADS):
        S[h * OUT_DIM:(h + 1) * OUT_DIM, h] = as32[h]
        S[h * OUT_DIM:(h + 1) * OUT_DIM, HEADS + h] = ad32[h]
    wad = (W32 @ S).astype(np.float16)          # param-only host matmul

    # launch 1: per-node attention stats
    nc1 = build_nc_stats(npc, N_CORES)
    in_maps1 = [dict(
        xT_slab=np.ascontiguousarray(x16[c * npc:(c + 1) * npc].T),
        wad_pd=wad) for c in range(N_CORES)]
    res1 = run_bass_kernel_spmd(nc1, in_maps1, core_ids=list(range(N_CORES)),
                                trace=trace)
    statv = np.concatenate(
        [res1.results[c]["statv"] for c in range(N_CORES)], axis=1)

    streams = build_streams(cores, statv)
    bias_nonzero = bool(np.any(bias32))
    kw = dict(cfg_kw or {})
    cfg = Cfg(npc, n_cores=N_CORES, bias_nonzero=bias_nonzero, **kw)

    nc2 = build_nc2(cfg, sched)
    iota = np.tile(np.arange(WIN, dtype=np.float16), (128, 1))
    biasb = np.tile(bias32, (128, 1)).astype(np.float32)
    wf16 = W32.astype(np.float16)
    in_maps = [dict(x16=x16, wf=wf16, iota16=iota, biasb=biasb,
                    streams=streams[c]) for c in range(N_CORES)]
    res = run_bass_kernel_spmd(nc2, in_maps, core_ids=list(range(N_CORES)),
                               trace=trace)
    out = np.empty((N_NODES, HEADS * OUT_DIM), np.float32)
    for c in range(N_CORES):
        dev = res.results[c]["out"]                 # [npad, HD] slot order
        perm = perms[c]
        valid = perm >= 0
        out[c * npc + perm[valid]] = dev[valid]
    parts = dict(nc1=nc1, in_maps1=in_maps1, nc2=nc2, in_maps2=in_maps,
                 res1=res1, res2=res, n_cores=N_CORES, cfg=cfg, npc=npc,
                 sched=sched)
    return out, parts


def make_pjrt_fn(nc, in_maps, n_cores):
    """Build a jitted PJRT executor for a prebuilt Bass module (axon path).
    Returns (fn, args); inputs are pre-staged on device."""
    import jax
    from jax.sharding import Mesh, NamedSharding, PartitionSpec
    from jax.experimental.shard_map import shard_map

    import concourse.mybir as mybir_
    from concourse import bass2jax as b2j

    b2j.install_neuronx_cc_hook()
    partition_name = (nc.partition_id_tensor.name
                      if nc.partition_id_tensor else None)
    in_names, out_names, out_avals, zero_outs = [], [], [], []
    for alloc in nc.m.functions[0].allocations:
        if not isinstance(alloc, mybir_.MemoryLocationSet):
            continue
        name = alloc.memorylocations[0].name
        if alloc.kind == "ExternalInput":
            if name != partition_name:
                in_names.append(name)
        elif alloc.kind == "ExternalOutput":
            dt = mybir_.dt.np(alloc.dtype)
            out_avals.append(jax.core.ShapedArray(tuple(alloc.tensor_shape), dt))
            out_names.append(name)
            zero_outs.append(np.zeros(tuple(alloc.tensor_shape), dt))

    # the bind's in_names must cover ALL operands (inputs + zero-out bufs
    # + partition id) — neuronx_cc_hook asserts len(in_names) == n_operands.
    bind_names = list(in_names) + list(out_names)
    if partition_name is not None:
        bind_names.append(partition_name)

    def _body(*args):
        operands = list(args)
        if partition_name is not None:
            operands.append(b2j.partition_id_tensor())
        outs = b2j._bass_exec_p.bind(
            *operands, out_avals=tuple(out_avals), in_names=tuple(bind_names),
            out_names=tuple(out_names), lowering_input_output_aliases=(),
            sim_require_finite=True, sim_require_nnan=True, nc=nc)
        return tuple(outs)

    n_params = len(in_names)
    devices = jax.devices()[:n_cores]
    mesh = Mesh(np.asarray(devices), ("core",))
    spec = PartitionSpec("core")
    fn = jax.jit(shard_map(_body, mesh=mesh,
                           in_specs=(spec,) * (n_params + len(zero_outs)),
                           out_specs=(spec,) * len(out_names),
                           check_rep=False), keep_unused=True)
    sh = NamedSharding(mesh, spec)
    args = [jax.device_put(
                np.concatenate([in_maps[c][nm] for c in range(n_cores)], 0), sh)
            for nm in in_names]
    args += [jax.device_put(
                np.zeros((n_cores * z.shape[0], *z.shape[1:]), z.dtype), sh)
             for z in zero_outs]
    return fn, args


def bench_pair(fnA, argsA, fnB, argsB, iters=24):
    """Interleaved wall-clock of two executables with a 4-byte D2H fetch as
    the completion sync (block_until_ready alone is lost in ~40-90ms axon RPC
    noise; interleaving + cluster-min cancels the shared offset)."""
    import time as _time
    for fn, args in ((fnA, argsA), (fnB, argsB)):
        r = fn(*args)
        _ = np.asarray(r[0][0:1, 0:1])
    pa, pb = [], []
    for _ in range(iters):
        t0 = _time.perf_counter()
        r = fnA(*argsA)
        _ = np.asarray(r[0][0:1, 0:1])
        pa.append(_time.perf_counter() - t0)
        t0 = _time.perf_counter()
        r = fnB(*argsB)
        _ = np.asarray(r[0][0:1, 0:1])
        pb.append(_time.perf_counter() - t0)
    return np.array(pa), np.array(pb)


def bench_slope(ncA, ncB, in_maps, n_cores, reps, iters=24):
    """HW ns of one kernel body via the repeat-slope method: ncA has repeat=1,
    ncB has repeat=reps; returns (per-rep seconds, raw pair arrays)."""
    fnA, argsA = make_pjrt_fn(ncA, in_maps, n_cores)
    fnB, argsB = make_pjrt_fn(ncB, in_maps, n_cores)
    pa, pb = bench_pair(fnA, argsA, fnB, argsB, iters=iters)
    # medians: robust against the occasional anomalous fast RPC round-trip
    per = (np.median(pb) - np.median(pa)) / (reps - 1)
    return max(0.0, per), (pa, pb)


def kernel(**inputs) -> np.ndarray:
    out, _ = run(inputs["x"], inputs["edge_index"], inputs["W"],
                 inputs["att_src"], inputs["att_dst"], inputs["bias"])
    return out


# revision 23
# speedup vs baseline: 2.2439x; 1.0387x over previous
# MixGAT layer (GATConv + beta-mix swish) on 8 Trainium2 NeuronCores, v2.
#
# Strategy (dst-node sharding):
#  - Nodes partitioned across 8 cores by dst id; each core owns N/8 dst rows.
#  - KEY CHANGE vs v1: aggregation is linear in xp = x @ W, so we aggregate
#    RAW x features per dst and project ONCE per dst node afterwards:
#      out[d] = (softmax-weighted-sum_e x[src_e]) / denom @ W
#    This removes the on-device projection-table build (old phase A): the
#    gather table is just x cast to f16 on the host (node-major, 256B rows
#    instead of 512B).
#  - Launch 1 (tiny): per-node attention stats a_src/a_dst = x @ (W@att) on
#    device; host expands them into per-edge streams (indexing only).
#  - Launch 2, per superblock (128 dst nodes = 4 fixed 32-node groups):
#      dma_gather x rows per edge slot (lo/hi int16-index split, 256B rows,
#      single-packet descriptors spread over 4 SWDGE queues),
#      expa = exp(lrelu(a_src+a_dst)) from streams,
#      Mw[e, h*32+c] = expa[e, h] * onehot(dst slot c),
#      per 128-edge block:  U2[feat, slot] += glo_blk(lhsT) @ Mw(rhs)
#                           Dt[slot, 1]   += Mw(lhsT) @ ones(rhs)
#      per group: project  Z[c, h*32+o] = U2[:, h*32..](lhsT) @ W[:, h*32..]
#      per sb: denominators to [c, g, h] via 4 partition-shifted copies,
#      normalize, beta-mix swish, ONE node-ordered output DMA.
#    Fixed 32-node groups keep outputs contiguous: no scratch roundtrip and
#    no permutation pass. Per-group block counts are padded to the max over
#    cores so one SPMD module serves all 8 cores.
#
# kernel(**inputs) is self-contained: preprocessing is pure numpy (sorting /
# indexing / dtype casts only), device kernels built with bass/Tile, run via
# run_bass_kernel_spmd on cores 0-7.

import numpy as np

import concourse.bass as bass
import concourse.mybir as mybir
import concourse.tile as tile
from concourse import bacc
from concourse.bass_utils import run_bass_kernel_spmd

F32 = mybir.dt.float32
F16 = mybir.dt.float16
I16 = mybir.dt.int16

# problem constants
N_NODES = 50000
IN_DIM = 128
HEADS = 4
OUT_DIM = 32
LEAKY_SLOPE = 0.2
BETA = 0.5
CMIX = 1.2
N_CORES = 8

# static schedule constants
WIN = 32          # dst nodes per group (PSUM slots = HEADS*WIN = 128)
BLK = 128         # edges per block (gather slots -> partitions)
GPB = 4           # groups per superblock (4*32 = 128 dst nodes)
SPLIT = 32768     # int16-addressable table split
DEAD = 100.0      # colidx value for dead slots (never equals iota 0..31)
GNJ = 1024        # rows per dma_gather call (SWDGE ring holds scratch/16
                  # descriptors; stay strictly under that at 1 desc/row)
NQ = 4            # SWDGE queues to spread gathers over
SCRATCH = 32768   # dynamic dma scratch (ring) bytes per partition
SINGLE_PACKET = True


def _wrap16(v):
    """idx vector [S*16] -> dma_gather idx layout [128, S]."""
    s = v.reshape(-1, 16).T                      # [16, S]
    return np.tile(s, (8, 1)).astype(np.int16)   # [128, S]


def _gather_chunks(total, gnj):
    out = []
    o = 0
    while o < total:
        c = min(gnj, total - o)
        out.append((o, c))
        o += c
    return out


class Cfg:
    def __init__(self, npc, n_cores=N_CORES, bias_nonzero=False, repeat=1,
                 blevel=4, gnj=GNJ, nq=NQ, scratch=SCRATCH, sp=SINGLE_PACKET,
                 poolpct=0):
        self.npc = npc
        self.n_cores = n_cores
        self.bias_nonzero = bias_nonzero
        self.repeat = repeat
        self.blevel = blevel   # 1 gather only; 2 +mw; 3 +matmul; 4 full
        self.gnj = gnj
        self.nq = nq
        self.scratch = scratch
        self.sp = sp
        self.poolpct = poolpct  # % of oneh/mw blocks built on GPSIMD


# ---------------------------------------------------------------- host side

def build_nc_stats(n_rows, n_cores, repeat=1):
    """Launch-1 mini kernel: statv[8, n_rows] = (W@[as|ad]).T @ xT_slab."""
    nc = bacc.Bacc("TRN2", target_bir_lowering=False, debug=False,
                   num_devices=n_cores)
    TW = 512
    H2 = 2 * HEADS
    xs_t = nc.dram_tensor("xT_slab", [IN_DIM, n_rows], F16, kind="ExternalInput")
    wad_t = nc.dram_tensor("wad_pd", [IN_DIM, H2], F16, kind="ExternalInput")
    out_t = nc.dram_tensor("statv", [H2, n_rows], F32, kind="ExternalOutput")
    with tile.TileContext(nc) as tc:
        with (tc.tile_pool(name="c", bufs=1) as cp,
              tc.tile_pool(name="s", bufs=3) as sp,
              tc.tile_pool(name="p2", bufs=3, space="PSUM") as pp2):
            wad_c = cp.tile([IN_DIM, H2], F16)
            nc.sync.dma_start(wad_c[:], wad_t.ap())
            for _rep in range(repeat):
                for n0 in range(0, n_rows, TW):
                    p = min(TW, n_rows - n0)
                    xt8 = sp.tile([128, TW], F16, tag="xt")
                    nc.sync.dma_start(xt8[:, :p], xs_t.ap()[:, n0:n0 + p])
                    av_ps = pp2.tile([H2, TW], F32, tag="av")
                    nc.tensor.matmul(av_ps[:, :p], lhsT=wad_c[:], rhs=xt8[:, :p],
                                     start=True, stop=True)
                    av8 = sp.tile([H2, TW], F32, tag="av8")
                    nc.vector.tensor_copy(av8[:, :p], av_ps[:, :p])
                    nc.sync.dma_start(out_t.ap()[:, n0:n0 + p], av8[:, :p])
    nc.compile()
    return nc


def preprocess(edge_index, n_all, npc, n_cores):
    """Static schedules: fixed 32-node groups, per-group block counts padded
    to the max over cores (one SPMD module). Pure numpy indexing."""
    src = np.asarray(edge_index[0], dtype=np.int64)
    dst = np.asarray(edge_index[1], dtype=np.int64)
    loop = np.arange(n_all, dtype=np.int64)
    src = np.concatenate([src, loop])
    dst = np.concatenate([dst, loop])
    order = np.argsort(dst, kind="stable")
    src = src[order]
    dst = dst[order]

    n_grp = (npc + WIN - 1) // WIN
    g_pad = ((n_grp + GPB - 1) // GPB) * GPB
    nsb = g_pad // GPB
    pad_n = g_pad * WIN - npc
    core_bounds = np.searchsorted(dst, np.arange(n_cores + 1) * npc)

    # stage A: per core, degree-balanced assignment of nodes to 32-node
    # groups (minimizes per-group block counts AND aligns them across
    # cores so the SPMD max-over-cores padding is tight), then per-group
    # lo/hi edge arrays. Device rows come out in group-slot order; run()
    # un-permutes on the host (indexing only).
    per_cg = []
    perms = []
    for c in range(n_cores):
        b0, b1 = core_bounds[c], core_bounds[c + 1]
        s = src[b0:b1]
        d = (dst[b0:b1] - c * npc).astype(np.int64)
        if pad_n:  # virtual degree-1 edges for pad slots
            s = np.concatenate([s, np.zeros(pad_n, dtype=np.int64)])
            d = np.concatenate([d, np.arange(npc, npc + pad_n, dtype=np.int64)])
        ntot = g_pad * WIN
        lo_m = s < SPLIT
        deg_lo = np.bincount(d[lo_m], minlength=ntot).astype(np.float64)
        deg_hi = np.bincount(d[~lo_m], minlength=ntot).astype(np.float64)
        G = g_pad
        cnt = np.zeros(G, np.int64)
        slo = np.zeros(G, np.int64)
        shi = np.zeros(G, np.int64)
        g_of = np.empty(ntot, np.int64)
        c_of = np.empty(ntot, np.int64)
        dl = deg_lo.astype(np.int64)
        dh = deg_hi.astype(np.int64)
        # greedy bin packing that directly minimizes block-count (ceil)
        # increments; groups end up filled to just under 128-multiples
        for n in np.argsort(-(dl + dh), kind="stable"):
            nlo, nhi = dl[n], dh[n]
            db = (((slo + nlo + BLK - 1) // BLK) - ((slo + BLK - 1) // BLK)
                  + ((shi + nhi + BLK - 1) // BLK) - ((shi + BLK - 1) // BLK))
            # secondary: prefer landing closest to a block boundary
            rem = ((-(slo + nlo)) % BLK) + ((-(shi + nhi)) % BLK)
            score = db * 1024 + (rem >> 3)
            score[cnt >= WIN] = 1 << 30
            g = int(np.argmin(score))
            g_of[n] = g
            c_of[n] = cnt[g]
            cnt[g] += 1
            slo[g] += nlo
            shi[g] += nhi
        # schedule slot k = k-th group by descending block needs (aligns
        # the per-slot maxima across cores)
        gorder = np.lexsort((-shi, -slo,
                             -((slo + BLK - 1) // BLK + (shi + BLK - 1) // BLK)))
        slot_of = np.empty(G, np.int64)
        slot_of[gorder] = np.arange(G)
        eg = slot_of[g_of[d]]                       # edge -> schedule slot
        order2 = np.lexsort((s, eg))                # slot-major, src-sorted
        s2, d2, eg2 = s[order2], d[order2], eg[order2]
        gb = np.searchsorted(eg2, np.arange(G + 1))
        rows = []
        for g in range(G):
            e0, e1 = gb[g], gb[g + 1]
            gs = s2[e0:e1]
            gc = c_of[d2[e0:e1]]
            gdst = np.minimum(c * npc + d2[e0:e1], n_all - 1)
            m = gs < SPLIT
            rows.append(((gs[m], gc[m], gdst[m]),
                         (gs[~m] - SPLIT, gc[~m], gdst[~m])))
        per_cg.append(rows)
        # perm[r]: device row r = slot k*WIN + c -> local node id (or -1)
        perm = np.full(ntot, -1, dtype=np.int64)
        node_rows = slot_of[g_of] * WIN + c_of      # node -> device row
        nodes = np.arange(ntot)
        perm[node_rows] = np.where(nodes < npc, nodes, -1)
        perms.append(perm)

    # stage B: global per-group block counts (max over cores)
    nlo_g = [max((len(per_cg[c][g][0][0]) + BLK - 1) // BLK
                 for c in range(n_cores)) for g in range(g_pad)]
    nhi_g = [max((len(per_cg[c][g][1][0]) + BLK - 1) // BLK
                 for c in range(n_cores)) for g in range(g_pad)]
    sched = []
    for sb in range(nsb):
        gs = range(sb * GPB, (sb + 1) * GPB)
        sched.append(([nlo_g[g] for g in gs], [nhi_g[g] for g in gs]))

    # stage C: per-core padded stream arrays
    def pad_block(vals, nblk, fill, dtype):
        a = np.full(nblk * BLK, fill, dtype=dtype)
        a[:len(vals)] = vals
        return a

    cores = []
    for c in range(n_cores):
        sbs = []
        for sb in range(nsb):
            gl = range(sb * GPB, (sb + 1) * GPB)
            idx_parts, col_parts, src_parts, dst_parts = [], [], [], []
            for half in (0, 1):
                cnt_g = nlo_g if half == 0 else nhi_g
                for g in gl:
                    hs, hc, hd = per_cg[c][g][half]
                    nb = cnt_g[g]
                    if nb == 0:
                        continue
                    idx_parts.append((half, pad_block(hs, nb, 0, np.int64)))
                    col_parts.append(pad_block(hc.astype(np.float16), nb,
                                               DEAD, np.float16))
                    src_parts.append(pad_block(
                        hs + (0 if half == 0 else SPLIT), nb, 0, np.int64))
                    dst_parts.append(pad_block(hd, nb, 0, np.int64))
            lo_idx = np.concatenate([a for h, a in idx_parts if h == 0]) \
                if any(h == 0 for h, _ in idx_parts) else np.zeros(0, np.int64)
            hi_idx = np.concatenate([a for h, a in idx_parts if h == 1]) \
                if any(h == 1 for h, _ in idx_parts) else np.zeros(0, np.int64)
            colidx = np.concatenate(col_parts).reshape(-1, BLK)   # [nbk,128]
            srcid = np.concatenate(src_parts).reshape(-1, BLK)
            dstid = np.concatenate(dst_parts).reshape(-1, BLK)
            sbs.append(dict(
                idx_lo=_wrap16(lo_idx) if len(lo_idx) else
                    np.zeros((128, 0), np.int16),
                idx_hi=_wrap16(hi_idx) if len(hi_idx) else
                    np.zeros((128, 0), np.int16),
                colidx=np.ascontiguousarray(colidx.T),            # [128,nbk]
                srcid=srcid, dstid=dstid))
        cores.append(sbs)
    return nsb, sched, cores, perms


def build_streams(cores, statv):
    """Per-edge a_src/a_dst expansion (indexing only) + packed stream blob."""
    asrcv, adstv = statv[:HEADS], statv[HEADS:]             # [4, n_all] f32
    outs = []
    for sbs in cores:
        blobs = []
        for sb in sbs:
            a_s = np.moveaxis(asrcv[:, sb["srcid"]], 0, -1)  # [nbk,128,4]
            a_d = np.moveaxis(adstv[:, sb["dstid"]], 0, -1)
            a8 = np.concatenate([a_s, a_d], axis=2)          # [nbk,128,8]
            a8 = np.ascontiguousarray(
                a8.transpose(1, 0, 2).astype(np.float16))    # [128,nbk,8]
            blobs.append(np.concatenate(
                [sb["idx_lo"], sb["idx_hi"], sb["colidx"].view(np.int16),
                 a8.reshape(128, -1).view(np.int16)], axis=1))
        outs.append(np.ascontiguousarray(np.concatenate(blobs, axis=1)))
    return outs


# -------------------------------------------------------------- device side

def build_nc2(cfg: Cfg, sched):
    nc = bacc.Bacc("TRN2", target_bir_lowering=False, debug=False,
                   num_devices=cfg.n_cores, num_swdge_queues=cfg.nq,
                   dynamic_dma_scratch_size=cfg.scratch)
    npc = cfg.npc
    HD = HEADS * OUT_DIM
    nsb = len(sched)
    nbk_s = [sum(l) + sum(h) for l, h in sched]
    TOT = sum(17 * b for b in nbk_s)
    nlo_max = max(sum(l) for l, _ in sched)
    nhi_max = max(sum(h) for _, h in sched)
    nbk_max = max(nbk_s)

    x_t = nc.dram_tensor("x16", [N_NODES, IN_DIM], F16, kind="ExternalInput")
    wf_t = nc.dram_tensor("wf", [IN_DIM, HD], F16, kind="ExternalInput")
    iota_t = nc.dram_tensor("iota16", [128, WIN], F16, kind="ExternalInput")
    biasb_t = nc.dram_tensor("biasb", [128, HD], F32, kind="ExternalInput")
    st_t = nc.dram_tensor("streams", [128, TOT], I16, kind="ExternalInput")
    npad = nsb * GPB * WIN
    out_t = nc.dram_tensor("out", [npad, HD], F32, kind="ExternalOutput")

    with tile.TileContext(nc) as tc:
        with tc.tile_pool(name="consts", bufs=1) as cpool:
            wf_c = cpool.tile([IN_DIM, HD], F16)
            nc.sync.dma_start(wf_c[:], wf_t.ap())
            iota_c = cpool.tile([128, WIN], F16)
            nc.sync.dma_start(iota_c[:], iota_t.ap())
            biasb_c = cpool.tile([128, HD], F32)
            nc.sync.dma_start(biasb_c[:], biasb_t.ap())
            ones_c = cpool.tile([128, 1], F16)
            nc.vector.memset(ones_c[:], 1.0)

            with (tc.tile_pool(name="pb_g", bufs=3) as gp,
                  tc.tile_pool(name="pb_m", bufs=3) as mp,
                  tc.tile_pool(name="pb_s", bufs=3) as sp,
                  tc.tile_pool(name="pb_z", bufs=2) as zp,
                  tc.tile_pool(name="pb_u", bufs=3, space="PSUM") as pu,
                  tc.tile_pool(name="pb_d", bufs=2, space="PSUM") as pdp,
                  tc.tile_pool(name="pb_w", bufs=3, space="PSUM") as pw):
                BL = cfg.blevel
                qi = 0
                for _rep in range(cfg.repeat):
                    off = 0
                    for sb in range(nsb):
                        nlo_l, nhi_l = sched[sb]
                        nlo, nhi = sum(nlo_l), sum(nhi_l)
                        nbk = nlo + nhi
                        W_sb = 17 * nbk
                        S0 = 8 * nlo
                        S1 = 8 * nbk
                        S2 = S1 + nbk
                        strm = sp.tile([128, 17 * nbk_max], I16, tag="strm")
                        nc.sync.dma_start(strm[:, :W_sb],
                                          st_t.ap()[:, off:off + W_sb])
                        off += W_sb
                        il = strm[:, 0:S0]
                        ih = strm[:, S0:S1]
                        cx = strm[:, S1:S2].bitcast(F16)
                        a8 = (strm[:, S2:W_sb].bitcast(F16)
                              .rearrange("p (b k) -> p b k", k=8))

                        glo = gp.tile([128, nlo_max, IN_DIM], F16, tag="glo")
                        for j0, nj in _gather_chunks(nlo * BLK, cfg.gnj):
                            nc.gpsimd.dma_gather(
                                glo[:, j0 // 128:(j0 + nj) // 128, :],
                                x_t.ap()[0:SPLIT, :],
                                il[:, j0 // 16:(j0 + nj) // 16],
                                nj, nj, IN_DIM, single_packet=cfg.sp,
                                queue_num=qi % cfg.nq)
                            qi += 1
                        ghi = gp.tile([128, nhi_max, IN_DIM], F16, tag="ghi")
                        for j0, nj in _gather_chunks(nhi * BLK, cfg.gnj):
                            nc.gpsimd.dma_gather(
                                ghi[:, j0 // 128:(j0 + nj) // 128, :],
                                x_t.ap()[SPLIT:N_NODES, :],
                                ih[:, j0 // 16:(j0 + nj) // 16],
                                nj, nj, IN_DIM, single_packet=cfg.sp,
                                queue_num=qi % cfg.nq)
                            qi += 1

                        if BL < 2:
                            continue
                        # expa = exp(lrelu(a_src + a_dst)), all f16
                        asum = sp.tile([128, nbk_max, HEADS], F16, tag="asum")
                        nc.vector.tensor_tensor(out=asum[:, :nbk, :],
                                                in0=a8[:, :, 0:HEADS],
                                                in1=a8[:, :, HEADS:8],
                                                op=mybir.AluOpType.add)
                        asc = sp.tile([128, nbk_max, HEADS], F16, tag="asc")
                        nc.vector.tensor_scalar(asc[:, :nbk, :],
                                                asum[:, :nbk, :], LEAKY_SLOPE,
                                                None, mybir.AluOpType.mult)
                        alr = sp.tile([128, nbk_max, HEADS], F16, tag="alr")
                        nc.vector.tensor_tensor(out=alr[:, :nbk, :],
                                                in0=asum[:, :nbk, :],
                                                in1=asc[:, :nbk, :],
                                                op=mybir.AluOpType.max)
                        expa = sp.tile([128, nbk_max, HEADS], F16, tag="expa")
                        nc.scalar.activation(expa[:, :nbk, :], alr[:, :nbk, :],
                                             mybir.ActivationFunctionType.Exp)
                        # onehot[e, b, c] = (iota[c] == colidx[e, b])
                        # Mw[e, b, h*32+c] = oneh * expa
                        # (built in two block-range chunks: head on DVE,
                        # tail on the otherwise-idle GPSIMD Q7 cores)
                        oneh = mp.tile([128, nbk_max, WIN], F16, tag="oneh")
                        mw = mp.tile([128, nbk_max, HEADS, WIN], F16, tag="mw")
                        nc.vector.tensor_tensor(
                            out=oneh[:, :nbk, :],
                            in0=iota_c[:].unsqueeze(1)
                                .to_broadcast([128, nbk, WIN]),
                            in1=cx.unsqueeze(2).to_broadcast([128, nbk, WIN]),
                            op=mybir.AluOpType.is_equal)
                        # mw multiply split: head on DVE, tail on the
                        # otherwise-idle GPSIMD (is_equal is DVE-only)
                        nsp = nbk - (nbk * cfg.poolpct) // 100
                        for b0, b1, eng in ((0, nsp, nc.vector),
                                            (nsp, nbk, nc.gpsimd)):
                            if b0 == b1:
                                continue
                            nb = b1 - b0
                            eng.tensor_tensor(
                                out=mw[:, b0:b1, :, :],
                                in0=oneh[:, b0:b1, :].unsqueeze(2)
                                    .to_broadcast([128, nb, HEADS, WIN]),
                                in1=expa[:, b0:b1, :].unsqueeze(3)
                                    .to_broadcast([128, nb, HEADS, WIN]),
                                op=mybir.AluOpType.mult)

                        if BL < 3:
                            continue
                        dt_ps = pdp.tile([128, GPB], F32, tag="dt")
                        zall_ps = pw.tile([WIN, GPB, HEADS, OUT_DIM], F32,
                                          tag="zall")
                        lo_c = np.cumsum([0] + nlo_l)
                        hi_c = np.cumsum([0] + nhi_l)
                        for g in range(GPB):
                            blocks = (
                                [(glo, lo_c[g] + j, lo_c[g] + j)
                                 for j in range(nlo_l[g])]
                                + [(ghi, hi_c[g] + j, nlo + hi_c[g] + j)
                                   for j in range(nhi_l[g])])
                            u2 = pu.tile([128, HD], F32, tag="u2")
                            for k, (gt, slot, bcol) in enumerate(blocks):
                                st = k == 0
                                sp_ = k == len(blocks) - 1
                                nc.tensor.matmul(u2[:],
                                                 lhsT=gt[:, slot, :],
                                                 rhs=mw[:, bcol, :, :],
                                                 start=st, stop=sp_)
                                nc.tensor.matmul(dt_ps[:, g:g + 1],
                                                 lhsT=mw[:, bcol, :, :],
                                                 rhs=ones_c[:],
                                                 start=st, stop=sp_)
                            if BL < 4:
                                continue
                            u2s = zp.tile([128, HD], F16, tag="u2s")
                            nc.scalar.activation(
                                u2s[:], u2[:],
                                mybir.ActivationFunctionType.Copy)
                            for h in range(HEADS):
                                nc.tensor.matmul(
                                    zall_ps[:, g, h, :],
                                    lhsT=u2s[:, h * WIN:(h + 1) * WIN],
                                    rhs=wf_c[:, h * OUT_DIM:(h + 1) * OUT_DIM],
                                    start=True, stop=True)
                        if BL < 4:
                            continue
                        # denominators -> [c, g, h]; normalize; swish; store
                        dts = zp.tile([128, GPB], F32, tag="dts")
                        nc.vector.tensor_copy(dts[:], dt_ps[:])
                        rec = zp.tile([WIN, GPB, HEADS], F32, tag="rec")
                        for h in range(HEADS):
                            nc.vector.tensor_copy(
                                rec[:, :, h], dts[h * WIN:(h + 1) * WIN, :])
                        nc.vector.reciprocal(rec[:], rec[:])
                        zn = zp.tile([WIN, GPB, HEADS, OUT_DIM], F32, tag="zn")
                        nc.vector.tensor_tensor(
                            out=zn[:], in0=zall_ps[:],
                            in1=rec[:].unsqueeze(3)
                                .to_broadcast([WIN, GPB, HEADS, OUT_DIM]),
                            op=mybir.AluOpType.mult)
                        if cfg.bias_nonzero:
                            nc.vector.tensor_tensor(
                                out=zn[:], in0=zn[:],
                                in1=biasb_c[0:WIN, :]
                                    .rearrange("c (h o) -> c h o", o=OUT_DIM)
                                    .unsqueeze(1)
                                    .to_broadcast([WIN, GPB, HEADS, OUT_DIM]),
                                op=mybir.AluOpType.add)
                        sg = zp.tile([WIN, GPB, HEADS, OUT_DIM], F16, tag="sg")
                        nc.scalar.activation(
                            sg[:], zn[:], mybir.ActivationFunctionType.Sigmoid)
                        mix = zp.tile([WIN, GPB, HEADS, OUT_DIM], F16,
                                      tag="mix")
                        nc.vector.tensor_scalar(mix[:], sg[:], CMIX - BETA,
                                                BETA, mybir.AluOpType.mult,
                                                mybir.AluOpType.add)
                        zrow = zp.tile([WIN, GPB, HEADS, OUT_DIM], F32,
                                       tag="zrow")
                        nc.vector.tensor_tensor(out=zrow[:], in0=zn[:],
                                                in1=mix[:],
                                                op=mybir.AluOpType.mult)
                        n0 = sb * GPB * WIN
                        nc.sync.dma_start(
                            out_t.ap()[n0:n0 + GPB * WIN, :]
                                 .rearrange("(g c) (h o) -> c g h o",
                                            c=WIN, o=OUT_DIM),
                            zrow[:])
    nc.compile()
    return nc


# ---------------------------------------------------------------- the API

def run(x, edge_index, W, att_src, att_dst, bias, trace=False, cfg_kw=None):
    npc = N_NODES // N_CORES
    nsb, sched, cores, perms = preprocess(edge_index, N_NODES, npc, N_CORES)

    x16 = np.asarray(x, np.float32).astype(np.float16)      # [N, 128]
    W32 = np.asarray(W, dtype=np.float32)
    as32 = np.asarray(att_src, dtype=np.float32)
    ad32 = np.asarray(att_dst, dtype=np.float32)
    bias32 = np.asarray(bias, dtype=np.float32)
    S = np.zeros((HEADS * OUT_DIM, 2 * HEADS), dtype=np.float32)
    for h in range(HEADS):
        S[h * OUT_DIM:(h + 1) * OUT_DIM, h] = as32[h]
        S[h * OUT_DIM:(h + 1) * OUT_DIM, HEADS + h] = ad32[h]
    wad = (W32 @ S).astype(np.float16)          # param-only host matmul

    # launch 1: per-node attention stats
    nc1 = build_nc_stats(npc, N_CORES)
    in_maps1 = [dict(
        xT_slab=np.ascontiguousarray(x16[c * npc:(c + 1) * npc].T),
        wad_pd=wad) for c in range(N_CORES)]
    res1 = run_bass_kernel_spmd(nc1, in_maps1, core_ids=list(range(N_CORES)),
                                trace=trace)
    statv = np.concatenate(
        [res1.results[c]["statv"] for c in range(N_CORES)], axis=1)

    streams = build_streams(cores, statv)
    bias_nonzero = bool(np.any(bias32))
    kw = dict(cfg_kw or {})
    cfg = Cfg(npc, n_cores=N_CORES, bias_nonzero=bias_nonzero, **kw)

    nc2 = build_nc2(cfg, sched)
    iota = np.tile(np.arange(WIN, dtype=np.float16), (128, 1))
    biasb = np.tile(bias32, (128, 1)).astype(np.float32)
    wf16 = W32.astype(np.float16)
    in_maps = [dict(x16=x16, wf=wf16, iota16=iota, biasb=biasb,
                    streams=streams[c]) for c in range(N_CORES)]
    res = run_bass_kernel_spmd(nc2, in_maps, core_ids=list(range(N_CORES)),
                               trace=trace)
    out = np.empty((N_NODES, HEADS * OUT_DIM), np.float32)
    for c in range(N_CORES):
        dev = res.results[c]["out"]                 # [npad, HD] slot order
        perm = perms[c]
        valid = perm >= 0
        out[c * npc + perm[valid]] = dev[valid]
    parts = dict(nc1=nc1, in_maps1=in_maps1, nc2=nc2, in_maps2=in_maps,
                 res1=res1, res2=res, n_cores=N_CORES, cfg=cfg, npc=npc,
                 sched=sched)
    return out, parts


def make_pjrt_fn(nc, in_maps, n_cores):
    """Build a jitted PJRT executor for a prebuilt Bass module (axon path).
    Returns (fn, args); inputs are pre-staged on device."""
    import jax
    from jax.sharding import Mesh, NamedSharding, PartitionSpec
    from jax.experimental.shard_map import shard_map

    import concourse.mybir as mybir_
    from concourse import bass2jax as b2j

    b2j.install_neuronx_cc_hook()
    partition_name = (nc.partition_id_tensor.name
                      if nc.partition_id_tensor else None)
    in_names, out_names, out_avals, zero_outs = [], [], [], []
    for alloc in nc.m.functions[0].allocations:
        if not isinstance(alloc, mybir_.MemoryLocationSet):
            continue
        name = alloc.memorylocations[0].name
        if alloc.kind == "ExternalInput":
            if name != partition_name:
                in_names.append(name)
        elif alloc.kind == "ExternalOutput":
            dt = mybir_.dt.np(alloc.dtype)
            out_avals.append(jax.core.ShapedArray(tuple(alloc.tensor_shape), dt))
            out_names.append(name)
            zero_outs.append(np.zeros(tuple(alloc.tensor_shape), dt))

    # the bind's in_names must cover ALL operands (inputs + zero-out bufs
    # + partition id) — neuronx_cc_hook asserts len(in_names) == n_operands.
    bind_names = list(in_names) + list(out_names)
    if partition_name is not None:
        bind_names.append(partition_name)

    def _body(*args):
        operands = list(args)
        if partition_name is not None:
            operands.append(b2j.partition_id_tensor())
        outs = b2j._bass_exec_p.bind(
            *operands, out_avals=tuple(out_avals), in_names=tuple(bind_names),
            out_names=tuple(out_names), lowering_input_output_aliases=(),
            sim_require_finite=True, sim_require_nnan=True, nc=nc)
        return tuple(outs)

    n_params = len(in_names)
    devices = jax.devices()[:n_cores]
    mesh = Mesh(np.asarray(devices), ("core",))
    spec = PartitionSpec("core")
    fn = jax.jit(shard_map(_body, mesh=mesh,
                           in_specs=(spec,) * (n_params + len(zero_outs)),
                           out_specs=(spec,) * len(out_names),
                           check_rep=False), keep_unused=True)
    sh = NamedSharding(mesh, spec)
    args = [jax.device_put(
                np.concatenate([in_maps[c][nm] for c in range(n_cores)], 0), sh)
            for nm in in_names]
    args += [jax.device_put(
                np.zeros((n_cores * z.shape[0], *z.shape[1:]), z.dtype), sh)
             for z in zero_outs]
    return fn, args


def bench_pair(fnA, argsA, fnB, argsB, iters=24):
    """Interleaved wall-clock of two executables with a 4-byte D2H fetch as
    the completion sync (block_until_ready alone is lost in ~40-90ms axon RPC
    noise; interleaving + cluster-min cancels the shared offset)."""
    import time as _time
    for fn, args in ((fnA, argsA), (fnB, argsB)):
        r = fn(*args)
        _ = np.asarray(r[0][0:1, 0:1])
    pa, pb = [], []
    for _ in range(iters):
        t0 = _time.perf_counter()
        r = fnA(*argsA)
        _ = np.asarray(r[0][0:1, 0:1])
        pa.append(_time.perf_counter() - t0)
        t0 = _time.perf_counter()
        r = fnB(*argsB)
        _ = np.asarray(r[0][0:1, 0:1])
        pb.append(_time.perf_counter() - t0)
    return np.array(pa), np.array(pb)


def bench_slope(ncA, ncB, in_maps, n_cores, reps, iters=24):
    """HW ns of one kernel body via the repeat-slope method: ncA has repeat=1,
    ncB has repeat=reps; returns (per-rep seconds, raw pair arrays)."""
    fnA, argsA = make_pjrt_fn(ncA, in_maps, n_cores)
    fnB, argsB = make_pjrt_fn(ncB, in_maps, n_cores)
    pa, pb = bench_pair(fnA, argsA, fnB, argsB, iters=iters)
    # medians: robust against the occasional anomalous fast RPC round-trip
    per = (np.median(pb) - np.median(pa)) / (reps - 1)
    return max(0.0, per), (pa, pb)


def kernel(**inputs) -> np.ndarray:
    out, _ = run(inputs["x"], inputs["edge_index"], inputs["W"],
                 inputs["att_src"], inputs["att_dst"], inputs["bias"])
    return out


# revision 31
# speedup vs baseline: 2.6161x; 1.1659x over previous
# MixGAT layer (GATConv + beta-mix swish) on 8 Trainium2 NeuronCores, v2.
#
# Strategy (dst-node sharding):
#  - Nodes partitioned across 8 cores by dst id; each core owns N/8 dst rows.
#  - KEY CHANGE vs v1: aggregation is linear in xp = x @ W, so we aggregate
#    RAW x features per dst and project ONCE per dst node afterwards:
#      out[d] = (softmax-weighted-sum_e x[src_e]) / denom @ W
#    This removes the on-device projection-table build (old phase A): the
#    gather table is just x cast to f16 on the host (node-major, 256B rows
#    instead of 512B).
#  - Launch 1 (tiny): per-node attention stats a_src/a_dst = x @ (W@att) on
#    device; host expands them into per-edge streams (indexing only).
#  - Launch 2, per superblock (128 dst nodes = 4 fixed 32-node groups):
#      dma_gather x rows per edge slot (lo/hi int16-index split, 256B rows,
#      single-packet descriptors spread over 4 SWDGE queues),
#      expa = exp(lrelu(a_src+a_dst)) from streams,
#      Mw[e, h*32+c] = expa[e, h] * onehot(dst slot c),
#      per 128-edge block:  U2[feat, slot] += glo_blk(lhsT) @ Mw(rhs)
#                           Dt[slot, 1]   += Mw(lhsT) @ ones(rhs)
#      per group: project  Z[c, h*32+o] = U2[:, h*32..](lhsT) @ W[:, h*32..]
#      per sb: denominators to [c, g, h] via 4 partition-shifted copies,
#      normalize, beta-mix swish, ONE node-ordered output DMA.
#    Fixed 32-node groups keep outputs contiguous: no scratch roundtrip and
#    no permutation pass. Per-group block counts are padded to the max over
#    cores so one SPMD module serves all 8 cores.
#
# kernel(**inputs) is self-contained: preprocessing is pure numpy (sorting /
# indexing / dtype casts only), device kernels built with bass/Tile, run via
# run_bass_kernel_spmd on cores 0-7.

import numpy as np

import concourse.bass as bass
import concourse.mybir as mybir
import concourse.tile as tile
from concourse import bacc
from concourse.bass_utils import run_bass_kernel_spmd

F32 = mybir.dt.float32
F16 = mybir.dt.float16
I16 = mybir.dt.int16

# problem constants
N_NODES = 50000
IN_DIM = 128
HEADS = 4
OUT_DIM = 32
LEAKY_SLOPE = 0.2
BETA = 0.5
CMIX = 1.2
N_CORES = 8

# static schedule constants
WIN = 32          # dst nodes per group (PSUM slots = HEADS*WIN = 128)
BLK = 128         # edges per block (gather slots -> partitions)
GPB = 4           # groups per superblock (4*32 = 128 dst nodes)
SPLIT = 32768     # int16-addressable table split
DEAD = 100.0      # colidx value for dead slots (never equals iota 0..31)
GNJ = 1024        # rows per dma_gather call (SWDGE ring holds scratch/16
                  # descriptors; stay strictly under that at 1 desc/row)
NQ = 4            # SWDGE queues to spread gathers over
SCRATCH = 32768   # dynamic dma scratch (ring) bytes per partition
SINGLE_PACKET = True


def _wrap16(v):
    """idx vector [S*16] -> dma_gather idx layout [128, S]."""
    s = v.reshape(-1, 16).T                      # [16, S]
    return np.tile(s, (8, 1)).astype(np.int16)   # [128, S]


def _gather_chunks(total, gnj):
    out = []
    o = 0
    while o < total:
        c = min(gnj, total - o)
        out.append((o, c))
        o += c
    return out


class Cfg:
    def __init__(self, npc, n_cores=N_CORES, bias_nonzero=False, repeat=1,
                 blevel=4, gnj=GNJ, nq=NQ, scratch=SCRATCH, sp=SINGLE_PACKET,
                 poolpct=0):
        self.npc = npc
        self.n_cores = n_cores
        self.bias_nonzero = bias_nonzero
        self.repeat = repeat
        self.blevel = blevel   # 1 gather only; 2 +mw; 3 +matmul; 4 full
        self.gnj = gnj
        self.nq = nq
        self.scratch = scratch
        self.sp = sp
        self.poolpct = poolpct  # % of oneh/mw blocks built on GPSIMD


# ---------------------------------------------------------------- host side

def build_nc_stats(n_rows, n_cores, repeat=1):
    """Launch-1 mini kernel: statv[8, n_rows] = (W@[as|ad]).T @ xT_slab."""
    nc = bacc.Bacc("TRN2", target_bir_lowering=False, debug=False,
                   num_devices=n_cores)
    TW = 512
    H2 = 2 * HEADS
    xs_t = nc.dram_tensor("xT_slab", [IN_DIM, n_rows], F16, kind="ExternalInput")
    wad_t = nc.dram_tensor("wad_pd", [IN_DIM, H2], F16, kind="ExternalInput")
    out_t = nc.dram_tensor("statv", [H2, n_rows], F32, kind="ExternalOutput")
    with tile.TileContext(nc) as tc:
        with (tc.tile_pool(name="c", bufs=1) as cp,
              tc.tile_pool(name="s", bufs=3) as sp,
              tc.tile_pool(name="p2", bufs=3, space="PSUM") as pp2):
            wad_c = cp.tile([IN_DIM, H2], F16)
            nc.sync.dma_start(wad_c[:], wad_t.ap())
            for _rep in range(repeat):
                for n0 in range(0, n_rows, TW):
                    p = min(TW, n_rows - n0)
                    xt8 = sp.tile([128, TW], F16, tag="xt")
                    nc.sync.dma_start(xt8[:, :p], xs_t.ap()[:, n0:n0 + p])
                    av_ps = pp2.tile([H2, TW], F32, tag="av")
                    nc.tensor.matmul(av_ps[:, :p], lhsT=wad_c[:], rhs=xt8[:, :p],
                                     start=True, stop=True)
                    av8 = sp.tile([H2, TW], F32, tag="av8")
                    nc.vector.tensor_copy(av8[:, :p], av_ps[:, :p])
                    nc.sync.dma_start(out_t.ap()[:, n0:n0 + p], av8[:, :p])
    nc.compile()
    return nc


def preprocess(edge_index, n_all, npc, n_cores):
    """Static schedules: fixed 32-node groups, per-group block counts padded
    to the max over cores (one SPMD module). Pure numpy indexing."""
    src = np.asarray(edge_index[0], dtype=np.int64)
    dst = np.asarray(edge_index[1], dtype=np.int64)
    loop = np.arange(n_all, dtype=np.int64)
    src = np.concatenate([src, loop])
    dst = np.concatenate([dst, loop])
    order = np.argsort(dst, kind="stable")
    src = src[order]
    dst = dst[order]

    n_grp = (npc + WIN - 1) // WIN
    g_pad = ((n_grp + GPB - 1) // GPB) * GPB
    nsb = g_pad // GPB
    pad_n = g_pad * WIN - npc
    core_bounds = np.searchsorted(dst, np.arange(n_cores + 1) * npc)

    # stage A: per core, degree-balanced assignment of nodes to 32-node
    # groups (minimizes per-group block counts AND aligns them across
    # cores so the SPMD max-over-cores padding is tight), then per-group
    # lo/hi edge arrays. Device rows come out in group-slot order; run()
    # un-permutes on the host (indexing only).
    per_cg = []
    perms = []
    for c in range(n_cores):
        b0, b1 = core_bounds[c], core_bounds[c + 1]
        s = src[b0:b1]
        d = (dst[b0:b1] - c * npc).astype(np.int64)
        if pad_n:  # virtual degree-1 edges for pad slots
            s = np.concatenate([s, np.zeros(pad_n, dtype=np.int64)])
            d = np.concatenate([d, np.arange(npc, npc + pad_n, dtype=np.int64)])
        ntot = g_pad * WIN
        lo_m = s < SPLIT
        deg_lo = np.bincount(d[lo_m], minlength=ntot).astype(np.float64)
        deg_hi = np.bincount(d[~lo_m], minlength=ntot).astype(np.float64)
        G = g_pad
        cnt = np.zeros(G, np.int64)
        slo = np.zeros(G, np.int64)
        shi = np.zeros(G, np.int64)
        g_of = np.empty(ntot, np.int64)
        c_of = np.empty(ntot, np.int64)
        dl = deg_lo.astype(np.int64)
        dh = deg_hi.astype(np.int64)
        # greedy bin packing that directly minimizes block-count (ceil)
        # increments; groups end up filled to just under 128-multiples
        for n in np.argsort(-(dl + dh), kind="stable"):
            nlo, nhi = dl[n], dh[n]
            db = (((slo + nlo + BLK - 1) // BLK) - ((slo + BLK - 1) // BLK)
                  + ((shi + nhi + BLK - 1) // BLK) - ((shi + BLK - 1) // BLK))
            # secondary: prefer landing closest to a block boundary
            rem = ((-(slo + nlo)) % BLK) + ((-(shi + nhi)) % BLK)
            score = db * 1024 + (rem >> 3)
            score[cnt >= WIN] = 1 << 30
            g = int(np.argmin(score))
            g_of[n] = g
            c_of[n] = cnt[g]
            cnt[g] += 1
            slo[g] += nlo
            shi[g] += nhi
        # schedule slot k = k-th group by descending block needs (aligns
        # the per-slot maxima across cores)
        gorder = np.lexsort((-shi, -slo,
                             -((slo + BLK - 1) // BLK + (shi + BLK - 1) // BLK)))
        slot_of = np.empty(G, np.int64)
        slot_of[gorder] = np.arange(G)
        eg = slot_of[g_of[d]]                       # edge -> schedule slot
        order2 = np.lexsort((s, eg))                # slot-major, src-sorted
        s2, d2, eg2 = s[order2], d[order2], eg[order2]
        gb = np.searchsorted(eg2, np.arange(G + 1))
        rows = []
        for g in range(G):
            e0, e1 = gb[g], gb[g + 1]
            gs = s2[e0:e1]
            gc = c_of[d2[e0:e1]]
            gdst = np.minimum(c * npc + d2[e0:e1], n_all - 1)
            m = gs < SPLIT
            rows.append(((gs[m], gc[m], gdst[m]),
                         (gs[~m] - SPLIT, gc[~m], gdst[~m])))
        per_cg.append(rows)
        # perm[r]: device row r = slot k*WIN + c -> local node id (or -1)
        perm = np.full(ntot, -1, dtype=np.int64)
        node_rows = slot_of[g_of] * WIN + c_of      # node -> device row
        nodes = np.arange(ntot)
        perm[node_rows] = np.where(nodes < npc, nodes, -1)
        perms.append(perm)

    # stage B: global per-group block counts (max over cores)
    nlo_g = [max((len(per_cg[c][g][0][0]) + BLK - 1) // BLK
                 for c in range(n_cores)) for g in range(g_pad)]
    nhi_g = [max((len(per_cg[c][g][1][0]) + BLK - 1) // BLK
                 for c in range(n_cores)) for g in range(g_pad)]
    sched = []
    for sb in range(nsb):
        gs = range(sb * GPB, (sb + 1) * GPB)
        sched.append(([nlo_g[g] for g in gs], [nhi_g[g] for g in gs]))

    # stage C: per-core padded stream arrays
    def pad_block(vals, nblk, fill, dtype):
        a = np.full(nblk * BLK, fill, dtype=dtype)
        a[:len(vals)] = vals
        return a

    cores = []
    for c in range(n_cores):
        sbs = []
        for sb in range(nsb):
            gl = range(sb * GPB, (sb + 1) * GPB)
            idx_parts, col_parts, src_parts, dst_parts = [], [], [], []
            for half in (0, 1):
                cnt_g = nlo_g if half == 0 else nhi_g
                for g in gl:
                    hs, hc, hd = per_cg[c][g][half]
                    nb = cnt_g[g]
                    if nb == 0:
                        continue
                    idx_parts.append((half, pad_block(hs, nb, 0, np.int64)))
                    col_parts.append(pad_block(hc.astype(np.float16), nb,
                                               DEAD, np.float16))
                    src_parts.append(pad_block(
                        hs + (0 if half == 0 else SPLIT), nb, 0, np.int64))
                    dst_parts.append(pad_block(hd, nb, 0, np.int64))
            lo_idx = np.concatenate([a for h, a in idx_parts if h == 0]) \
                if any(h == 0 for h, _ in idx_parts) else np.zeros(0, np.int64)
            hi_idx = np.concatenate([a for h, a in idx_parts if h == 1]) \
                if any(h == 1 for h, _ in idx_parts) else np.zeros(0, np.int64)
            colidx = np.concatenate(col_parts).reshape(-1, BLK)   # [nbk,128]
            srcid = np.concatenate(src_parts).reshape(-1, BLK)
            dstid = np.concatenate(dst_parts).reshape(-1, BLK)
            sbs.append(dict(
                idx_lo=_wrap16(lo_idx) if len(lo_idx) else
                    np.zeros((128, 0), np.int16),
                idx_hi=_wrap16(hi_idx) if len(hi_idx) else
                    np.zeros((128, 0), np.int16),
                colidx=np.ascontiguousarray(colidx.T),            # [128,nbk]
                srcid=srcid, dstid=dstid))
        cores.append(sbs)
    return nsb, sched, cores, perms


def build_streams(cores, statv):
    """Per-edge a_src/a_dst expansion (indexing only) + packed stream blob."""
    asrcv, adstv = statv[:HEADS], statv[HEADS:]             # [4, n_all] f32
    outs = []
    for sbs in cores:
        blobs = []
        for sb in sbs:
            a_s = np.moveaxis(asrcv[:, sb["srcid"]], 0, -1)  # [nbk,128,4]
            a_d = np.moveaxis(adstv[:, sb["dstid"]], 0, -1)
            a8 = np.concatenate([a_s, a_d], axis=2)          # [nbk,128,8]
            a8 = np.ascontiguousarray(
                a8.transpose(1, 0, 2).astype(np.float16))    # [128,nbk,8]
            # colidx replicated WIN x: the on-device one-hot compare then
            # has stride-1 operands only, enabling the DVE 2x f16 mode
            cxe = np.repeat(sb["colidx"][:, :, None], WIN, axis=2)
            blobs.append(np.concatenate(
                [sb["idx_lo"], sb["idx_hi"],
                 cxe.reshape(128, -1).view(np.int16),
                 a8.reshape(128, -1).view(np.int16)], axis=1))
        outs.append(np.ascontiguousarray(np.concatenate(blobs, axis=1)))
    return outs


# -------------------------------------------------------------- device side

def build_nc2(cfg: Cfg, sched):
    nc = bacc.Bacc("TRN2", target_bir_lowering=False, debug=False,
                   num_devices=cfg.n_cores, num_swdge_queues=cfg.nq,
                   dynamic_dma_scratch_size=cfg.scratch)
    npc = cfg.npc
    HD = HEADS * OUT_DIM
    nsb = len(sched)
    nbk_s = [sum(l) + sum(h) for l, h in sched]
    TOT = sum((8 + WIN + 8) * b for b in nbk_s)
    nlo_max = max(sum(l) for l, _ in sched)
    nhi_max = max(sum(h) for _, h in sched)
    nbk_max = max(nbk_s)

    x_t = nc.dram_tensor("x16", [N_NODES, IN_DIM], F16, kind="ExternalInput")
    wf_t = nc.dram_tensor("wf", [IN_DIM, HD], F16, kind="ExternalInput")
    iota_t = nc.dram_tensor("iota16", [128, WIN], F16, kind="ExternalInput")
    biasb_t = nc.dram_tensor("biasb", [128, HD], F32, kind="ExternalInput")
    st_t = nc.dram_tensor("streams", [128, TOT], I16, kind="ExternalInput")
    npad = nsb * GPB * WIN
    out_t = nc.dram_tensor("out", [npad, HD], F16, kind="ExternalOutput")

    with tile.TileContext(nc) as tc:
        with tc.tile_pool(name="consts", bufs=1) as cpool:
            wf_c = cpool.tile([IN_DIM, HD], F16)
            nc.sync.dma_start(wf_c[:], wf_t.ap())
            iota_c = cpool.tile([128, WIN], F16)
            nc.sync.dma_start(iota_c[:], iota_t.ap())
            biasb_c = cpool.tile([128, HD], F32)
            nc.sync.dma_start(biasb_c[:], biasb_t.ap())
            ones_c = cpool.tile([128, 1], F16)
            nc.vector.memset(ones_c[:], 1.0)

            with (tc.tile_pool(name="pb_g", bufs=3) as gp,
                  tc.tile_pool(name="pb_m", bufs=3) as mp,
                  tc.tile_pool(name="pb_s", bufs=3) as sp,
                  tc.tile_pool(name="pb_z", bufs=2) as zp,
                  tc.tile_pool(name="pb_u", bufs=3, space="PSUM") as pu,
                  tc.tile_pool(name="pb_d", bufs=2, space="PSUM") as pdp,
                  tc.tile_pool(name="pb_w", bufs=3, space="PSUM") as pw):
                BL = cfg.blevel
                qi = 0
                for _rep in range(cfg.repeat):
                    off = 0
                    for sb in range(nsb):
                        nlo_l, nhi_l = sched[sb]
                        nlo, nhi = sum(nlo_l), sum(nhi_l)
                        nbk = nlo + nhi
                        W_sb = (8 + WIN + 8) * nbk
                        S0 = 8 * nlo
                        S1 = 8 * nbk
                        S2 = S1 + WIN * nbk
                        strm = sp.tile([128, (8 + WIN + 8) * nbk_max], I16,
                                       tag="strm")
                        nc.sync.dma_start(strm[:, :W_sb],
                                          st_t.ap()[:, off:off + W_sb])
                        off += W_sb
                        il = strm[:, 0:S0]
                        ih = strm[:, S0:S1]
                        cx = (strm[:, S1:S2].bitcast(F16)
                              .rearrange("p (b c) -> p b c", c=WIN))
                        a8 = (strm[:, S2:W_sb].bitcast(F16)
                              .rearrange("p (b k) -> p b k", k=8))

                        glo = gp.tile([128, nlo_max, IN_DIM], F16, tag="glo")
                        for j0, nj in _gather_chunks(nlo * BLK, cfg.gnj):
                            nc.gpsimd.dma_gather(
                                glo[:, j0 // 128:(j0 + nj) // 128, :],
                                x_t.ap()[0:SPLIT, :],
                                il[:, j0 // 16:(j0 + nj) // 16],
                                nj, nj, IN_DIM, single_packet=cfg.sp,
                                queue_num=qi % cfg.nq)
                            qi += 1
                        ghi = gp.tile([128, nhi_max, IN_DIM], F16, tag="ghi")
                        for j0, nj in _gather_chunks(nhi * BLK, cfg.gnj):
                            nc.gpsimd.dma_gather(
                                ghi[:, j0 // 128:(j0 + nj) // 128, :],
                                x_t.ap()[SPLIT:N_NODES, :],
                                ih[:, j0 // 16:(j0 + nj) // 16],
                                nj, nj, IN_DIM, single_packet=cfg.sp,
                                queue_num=qi % cfg.nq)
                            qi += 1

                        if BL < 2:
                            continue
                        # expa = exp(lrelu(a_src + a_dst)), all f16
                        asum = sp.tile([128, nbk_max, HEADS], F16, tag="asum")
                        nc.vector.tensor_tensor(out=asum[:, :nbk, :],
                                                in0=a8[:, :, 0:HEADS],
                                                in1=a8[:, :, HEADS:8],
                                                op=mybir.AluOpType.add)
                        alr = sp.tile([128, nbk_max, HEADS], F16, tag="alr")
                        nc.vector.scalar_tensor_tensor(
                            out=alr[:, :nbk, :], in0=asum[:, :nbk, :],
                            scalar=LEAKY_SLOPE, in1=asum[:, :nbk, :],
                            op0=mybir.AluOpType.mult, op1=mybir.AluOpType.max)
                        expa = sp.tile([128, nbk_max, HEADS], F16, tag="expa")
                        nc.scalar.activation(expa[:, :nbk, :], alr[:, :nbk, :],
                                             mybir.ActivationFunctionType.Exp)
                        # onehot[e, b, c] = (iota[c] == colidx[e, b])
                        # Mw[e, b, h*32+c] = oneh * expa
                        # (built in two block-range chunks: head on DVE,
                        # tail on the otherwise-idle GPSIMD Q7 cores)
                        oneh = mp.tile([128, nbk_max, WIN], F16, tag="oneh")
                        mw = mp.tile([128, nbk_max, HEADS, WIN], F16, tag="mw")
                        nc.vector.tensor_tensor(
                            out=oneh[:, :nbk, :],
                            in0=iota_c[:].unsqueeze(1)
                                .to_broadcast([128, nbk, WIN]),
                            in1=cx,
                            op=mybir.AluOpType.is_equal)
                        # mw multiply split: head on DVE, tail on the
                        # otherwise-idle GPSIMD (is_equal is DVE-only)
                        nsp = nbk - (nbk * cfg.poolpct) // 100
                        for b0, b1, eng in ((0, nsp, nc.vector),
                                            (nsp, nbk, nc.gpsimd)):
                            if b0 == b1:
                                continue
                            nb = b1 - b0
                            eng.tensor_tensor(
                                out=mw[:, b0:b1, :, :],
                                in0=oneh[:, b0:b1, :].unsqueeze(2)
                                    .to_broadcast([128, nb, HEADS, WIN]),
                                in1=expa[:, b0:b1, :].unsqueeze(3)
                                    .to_broadcast([128, nb, HEADS, WIN]),
                                op=mybir.AluOpType.mult)

                        if BL < 3:
                            continue
                        dt_ps = pdp.tile([128, GPB], F32, tag="dt")
                        zall_ps = pw.tile([WIN, GPB, HEADS, OUT_DIM], F32,
                                          tag="zall")
                        lo_c = np.cumsum([0] + nlo_l)
                        hi_c = np.cumsum([0] + nhi_l)
                        for g in range(GPB):
                            blocks = (
                                [(glo, lo_c[g] + j, lo_c[g] + j)
                                 for j in range(nlo_l[g])]
                                + [(ghi, hi_c[g] + j, nlo + hi_c[g] + j)
                                   for j in range(nhi_l[g])])
                            u2 = pu.tile([128, HD], F32, tag="u2")
                            for k, (gt, slot, bcol) in enumerate(blocks):
                                st = k == 0
                                sp_ = k == len(blocks) - 1
                                nc.tensor.matmul(u2[:],
                                                 lhsT=gt[:, slot, :],
                                                 rhs=mw[:, bcol, :, :],
                                                 start=st, stop=sp_)
                                nc.tensor.matmul(dt_ps[:, g:g + 1],
                                                 lhsT=mw[:, bcol, :, :],
                                                 rhs=ones_c[:],
                                                 start=st, stop=sp_)
                            if BL < 4:
                                continue
                            u2s = zp.tile([128, HD], F16, tag="u2s")
                            nc.scalar.activation(
                                u2s[:], u2[:],
                                mybir.ActivationFunctionType.Copy)
                            for h in range(HEADS):
                                nc.tensor.matmul(
                                    zall_ps[:, g, h, :],
                                    lhsT=u2s[:, h * WIN:(h + 1) * WIN],
                                    rhs=wf_c[:, h * OUT_DIM:(h + 1) * OUT_DIM],
                                    start=True, stop=True)
                        if BL < 4:
                            continue
                        # denominators -> [c, g, h]; normalize; swish; store
                        dts = zp.tile([128, GPB], F32, tag="dts")
                        nc.scalar.copy(dts[:], dt_ps[:])
                        rec = zp.tile([WIN, GPB, HEADS], F32, tag="rec")
                        for h in range(HEADS):
                            nc.scalar.copy(
                                rec[:, :, h], dts[h * WIN:(h + 1) * WIN, :])
                        nc.vector.reciprocal(rec[:], rec[:])
                        zn = zp.tile([WIN, GPB, HEADS, OUT_DIM], F16, tag="zn")
                        nc.vector.tensor_tensor(
                            out=zn[:], in0=zall_ps[:],
                            in1=rec[:].unsqueeze(3)
                                .to_broadcast([WIN, GPB, HEADS, OUT_DIM]),
                            op=mybir.AluOpType.mult)
                        if cfg.bias_nonzero:
                            nc.vector.tensor_tensor(
                                out=zn[:], in0=zn[:],
                                in1=biasb_c[0:WIN, :]
                                    .rearrange("c (h o) -> c h o", o=OUT_DIM)
                                    .unsqueeze(1)
                                    .to_broadcast([WIN, GPB, HEADS, OUT_DIM]),
                                op=mybir.AluOpType.add)
                        sg = zp.tile([WIN, GPB, HEADS, OUT_DIM], F16, tag="sg")
                        nc.scalar.activation(
                            sg[:], zn[:], mybir.ActivationFunctionType.Sigmoid)
                        mix = zp.tile([WIN, GPB, HEADS, OUT_DIM], F16,
                                      tag="mix")
                        nc.vector.tensor_scalar(mix[:], sg[:], CMIX - BETA,
                                                BETA, mybir.AluOpType.mult,
                                                mybir.AluOpType.add)
                        zrow = zp.tile([WIN, GPB, HEADS, OUT_DIM], F16,
                                       tag="zrow")
                        nc.vector.tensor_tensor(out=zrow[:], in0=zn[:],
                                                in1=mix[:],
                                                op=mybir.AluOpType.mult)
                        n0 = sb * GPB * WIN
                        nc.sync.dma_start(
                            out_t.ap()[n0:n0 + GPB * WIN, :]
                                 .rearrange("(g c) (h o) -> c g h o",
                                            c=WIN, o=OUT_DIM),
                            zrow[:])
    nc.compile()
    return nc


# ---------------------------------------------------------------- the API

def run(x, edge_index, W, att_src, att_dst, bias, trace=False, cfg_kw=None):
    npc = N_NODES // N_CORES
    nsb, sched, cores, perms = preprocess(edge_index, N_NODES, npc, N_CORES)

    x16 = np.asarray(x, np.float32).astype(np.float16)      # [N, 128]
    W32 = np.asarray(W, dtype=np.float32)
    as32 = np.asarray(att_src, dtype=np.float32)
    ad32 = np.asarray(att_dst, dtype=np.float32)
    bias32 = np.asarray(bias, dtype=np.float32)
    S = np.zeros((HEADS * OUT_DIM, 2 * HEADS), dtype=np.float32)
    for h in range(HEADS):
        S[h * OUT_DIM:(h + 1) * OUT_DIM, h] = as32[h]
        S[h * OUT_DIM:(h + 1) * OUT_DIM, HEADS + h] = ad32[h]
    wad = (W32 @ S).astype(np.float16)          # param-only host matmul

    # launch 1: per-node attention stats
    nc1 = build_nc_stats(npc, N_CORES)
    in_maps1 = [dict(
        xT_slab=np.ascontiguousarray(x16[c * npc:(c + 1) * npc].T),
        wad_pd=wad) for c in range(N_CORES)]
    res1 = run_bass_kernel_spmd(nc1, in_maps1, core_ids=list(range(N_CORES)),
                                trace=trace)
    statv = np.concatenate(
        [res1.results[c]["statv"] for c in range(N_CORES)], axis=1)

    streams = build_streams(cores, statv)
    bias_nonzero = bool(np.any(bias32))
    kw = dict(cfg_kw or {})
    cfg = Cfg(npc, n_cores=N_CORES, bias_nonzero=bias_nonzero, **kw)

    nc2 = build_nc2(cfg, sched)
    iota = np.tile(np.arange(WIN, dtype=np.float16), (128, 1))
    biasb = np.tile(bias32, (128, 1)).astype(np.float32)
    wf16 = W32.astype(np.float16)
    in_maps = [dict(x16=x16, wf=wf16, iota16=iota, biasb=biasb,
                    streams=streams[c]) for c in range(N_CORES)]
    res = run_bass_kernel_spmd(nc2, in_maps, core_ids=list(range(N_CORES)),
                               trace=trace)
    out = np.empty((N_NODES, HEADS * OUT_DIM), np.float32)
    for c in range(N_CORES):
        dev = res.results[c]["out"]                 # [npad, HD] slot order
        perm = perms[c]
        valid = perm >= 0
        out[c * npc + perm[valid]] = dev[valid]
    parts = dict(nc1=nc1, in_maps1=in_maps1, nc2=nc2, in_maps2=in_maps,
                 res1=res1, res2=res, n_cores=N_CORES, cfg=cfg, npc=npc,
                 sched=sched)
    return out, parts


def make_pjrt_fn(nc, in_maps, n_cores):
    """Build a jitted PJRT executor for a prebuilt Bass module (axon path).
    Returns (fn, args); inputs are pre-staged on device."""
    import jax
    from jax.sharding import Mesh, NamedSharding, PartitionSpec
    from jax.experimental.shard_map import shard_map

    import concourse.mybir as mybir_
    from concourse import bass2jax as b2j

    b2j.install_neuronx_cc_hook()
    partition_name = (nc.partition_id_tensor.name
                      if nc.partition_id_tensor else None)
    in_names, out_names, out_avals, zero_outs = [], [], [], []
    for alloc in nc.m.functions[0].allocations:
        if not isinstance(alloc, mybir_.MemoryLocationSet):
            continue
        name = alloc.memorylocations[0].name
        if alloc.kind == "ExternalInput":
            if name != partition_name:
                in_names.append(name)
        elif alloc.kind == "ExternalOutput":
            dt = mybir_.dt.np(alloc.dtype)
            out_avals.append(jax.core.ShapedArray(tuple(alloc.tensor_shape), dt))
            out_names.append(name)
            zero_outs.append(np.zeros(tuple(alloc.tensor_shape), dt))

    # the bind's in_names must cover ALL operands (inputs + zero-out bufs
    # + partition id) — neuronx_cc_hook asserts len(in_names) == n_operands.
    bind_names = list(in_names) + list(out_names)
    if partition_name is not None:
        bind_names.append(partition_name)

    def _body(*args):
        operands = list(args)
        if partition_name is not None:
            operands.append(b2j.partition_id_tensor())
        outs = b2j._bass_exec_p.bind(
            *operands, out_avals=tuple(out_avals), in_names=tuple(bind_names),
            out_names=tuple(out_names), lowering_input_output_aliases=(),
            sim_require_finite=True, sim_require_nnan=True, nc=nc)
        return tuple(outs)

    n_params = len(in_names)
    devices = jax.devices()[:n_cores]
    mesh = Mesh(np.asarray(devices), ("core",))
    spec = PartitionSpec("core")
    fn = jax.jit(shard_map(_body, mesh=mesh,
                           in_specs=(spec,) * (n_params + len(zero_outs)),
                           out_specs=(spec,) * len(out_names),
                           check_rep=False), keep_unused=True)
    sh = NamedSharding(mesh, spec)
    args = [jax.device_put(
                np.concatenate([in_maps[c][nm] for c in range(n_cores)], 0), sh)
            for nm in in_names]
    args += [jax.device_put(
                np.zeros((n_cores * z.shape[0], *z.shape[1:]), z.dtype), sh)
             for z in zero_outs]
    return fn, args


def bench_pair(fnA, argsA, fnB, argsB, iters=24):
    """Interleaved wall-clock of two executables with a 4-byte D2H fetch as
    the completion sync (block_until_ready alone is lost in ~40-90ms axon RPC
    noise; interleaving + cluster-min cancels the shared offset)."""
    import time as _time
    for fn, args in ((fnA, argsA), (fnB, argsB)):
        r = fn(*args)
        _ = np.asarray(r[0][0:1, 0:1])
    pa, pb = [], []
    for _ in range(iters):
        t0 = _time.perf_counter()
        r = fnA(*argsA)
        _ = np.asarray(r[0][0:1, 0:1])
        pa.append(_time.perf_counter() - t0)
        t0 = _time.perf_counter()
        r = fnB(*argsB)
        _ = np.asarray(r[0][0:1, 0:1])
        pb.append(_time.perf_counter() - t0)
    return np.array(pa), np.array(pb)


def bench_slope(ncA, ncB, in_maps, n_cores, reps, iters=24):
    """HW ns of one kernel body via the repeat-slope method: ncA has repeat=1,
    ncB has repeat=reps; returns (per-rep seconds, raw pair arrays)."""
    fnA, argsA = make_pjrt_fn(ncA, in_maps, n_cores)
    fnB, argsB = make_pjrt_fn(ncB, in_maps, n_cores)
    pa, pb = bench_pair(fnA, argsA, fnB, argsB, iters=iters)
    # medians: robust against the occasional anomalous fast RPC round-trip
    per = (np.median(pb) - np.median(pa)) / (reps - 1)
    return max(0.0, per), (pa, pb)


def kernel(**inputs) -> np.ndarray:
    out, _ = run(inputs["x"], inputs["edge_index"], inputs["W"],
                 inputs["att_src"], inputs["att_dst"], inputs["bias"])
    return out


# revision 32
# speedup vs baseline: 3.2695x; 1.2498x over previous
# MixGAT layer (GATConv + beta-mix swish) on 8 Trainium2 NeuronCores, v2.
#
# Strategy (dst-node sharding):
#  - Nodes partitioned across 8 cores by dst id; each core owns N/8 dst rows.
#  - KEY CHANGE vs v1: aggregation is linear in xp = x @ W, so we aggregate
#    RAW x features per dst and project ONCE per dst node afterwards:
#      out[d] = (softmax-weighted-sum_e x[src_e]) / denom @ W
#    This removes the on-device projection-table build (old phase A): the
#    gather table is just x cast to f16 on the host (node-major, 256B rows
#    instead of 512B).
#  - Launch 1 (tiny): per-node attention stats a_src/a_dst = x @ (W@att) on
#    device; host expands them into per-edge streams (indexing only).
#  - Launch 2, per superblock (128 dst nodes = 4 fixed 32-node groups):
#      dma_gather x rows per edge slot (lo/hi int16-index split, 256B rows,
#      single-packet descriptors spread over 4 SWDGE queues),
#      expa = exp(lrelu(a_src+a_dst)) from streams,
#      Mw[e, h*32+c] = expa[e, h] * onehot(dst slot c),
#      per 128-edge block:  U2[feat, slot] += glo_blk(lhsT) @ Mw(rhs)
#                           Dt[slot, 1]   += Mw(lhsT) @ ones(rhs)
#      per group: project  Z[c, h*32+o] = U2[:, h*32..](lhsT) @ W[:, h*32..]
#      per sb: denominators to [c, g, h] via 4 partition-shifted copies,
#      normalize, beta-mix swish, ONE node-ordered output DMA.
#    Fixed 32-node groups keep outputs contiguous: no scratch roundtrip and
#    no permutation pass. Per-group block counts are padded to the max over
#    cores so one SPMD module serves all 8 cores.
#
# kernel(**inputs) is self-contained: preprocessing is pure numpy (sorting /
# indexing / dtype casts only), device kernels built with bass/Tile, run via
# run_bass_kernel_spmd on cores 0-7.

import numpy as np

import concourse.bass as bass
import concourse.mybir as mybir
import concourse.tile as tile
from concourse import bacc
from concourse.bass_utils import run_bass_kernel_spmd

F32 = mybir.dt.float32
F16 = mybir.dt.float16
I16 = mybir.dt.int16

# problem constants
N_NODES = 50000
IN_DIM = 128
HEADS = 4
OUT_DIM = 32
LEAKY_SLOPE = 0.2
BETA = 0.5
CMIX = 1.2
N_CORES = 8

# static schedule constants
WIN = 32          # dst nodes per group (PSUM slots = HEADS*WIN = 128)
BLK = 128         # edges per block (gather slots -> partitions)
GPB = 4           # groups per superblock (4*32 = 128 dst nodes)
SPLIT = 32768     # int16-addressable table split
DEAD = 100.0      # colidx value for dead slots (never equals iota 0..31)
GNJ = 1024        # rows per dma_gather call (SWDGE ring holds scratch/16
                  # descriptors; stay strictly under that at 1 desc/row)
NQ = 4            # SWDGE queues to spread gathers over
SCRATCH = 32768   # dynamic dma scratch (ring) bytes per partition
SINGLE_PACKET = True


def _wrap16(v):
    """idx vector [S*16] -> dma_gather idx layout [128, S]."""
    s = v.reshape(-1, 16).T                      # [16, S]
    return np.tile(s, (8, 1)).astype(np.int16)   # [128, S]


def _gather_chunks(total, gnj):
    out = []
    o = 0
    while o < total:
        c = min(gnj, total - o)
        out.append((o, c))
        o += c
    return out


class Cfg:
    def __init__(self, npc, n_cores=N_CORES, bias_nonzero=False, repeat=1,
                 blevel=4, gnj=GNJ, nq=NQ, scratch=SCRATCH, sp=SINGLE_PACKET,
                 poolpct=0):
        self.npc = npc
        self.n_cores = n_cores
        self.bias_nonzero = bias_nonzero
        self.repeat = repeat
        self.blevel = blevel   # 1 gather only; 2 +mw; 3 +matmul; 4 full
        self.gnj = gnj
        self.nq = nq
        self.scratch = scratch
        self.sp = sp
        self.poolpct = poolpct  # % of oneh/mw blocks built on GPSIMD


# ---------------------------------------------------------------- host side

def build_nc_stats(n_rows, n_cores, repeat=1):
    """Launch-1 mini kernel: statv[8, n_rows] = (W@[as|ad]).T @ xT_slab."""
    nc = bacc.Bacc("TRN2", target_bir_lowering=False, debug=False,
                   num_devices=n_cores)
    TW = 512
    H2 = 2 * HEADS
    xs_t = nc.dram_tensor("xT_slab", [IN_DIM, n_rows], F16, kind="ExternalInput")
    wad_t = nc.dram_tensor("wad_pd", [IN_DIM, H2], F16, kind="ExternalInput")
    out_t = nc.dram_tensor("statv", [H2, n_rows], F32, kind="ExternalOutput")
    with tile.TileContext(nc) as tc:
        with (tc.tile_pool(name="c", bufs=1) as cp,
              tc.tile_pool(name="s", bufs=3) as sp,
              tc.tile_pool(name="p2", bufs=3, space="PSUM") as pp2):
            wad_c = cp.tile([IN_DIM, H2], F16)
            nc.sync.dma_start(wad_c[:], wad_t.ap())
            for _rep in range(repeat):
                for n0 in range(0, n_rows, TW):
                    p = min(TW, n_rows - n0)
                    xt8 = sp.tile([128, TW], F16, tag="xt")
                    nc.sync.dma_start(xt8[:, :p], xs_t.ap()[:, n0:n0 + p])
                    av_ps = pp2.tile([H2, TW], F32, tag="av")
                    nc.tensor.matmul(av_ps[:, :p], lhsT=wad_c[:], rhs=xt8[:, :p],
                                     start=True, stop=True)
                    av8 = sp.tile([H2, TW], F32, tag="av8")
                    nc.vector.tensor_copy(av8[:, :p], av_ps[:, :p])
                    nc.sync.dma_start(out_t.ap()[:, n0:n0 + p], av8[:, :p])
    nc.compile()
    return nc


def preprocess(edge_index, n_all, npc, n_cores):
    """Static schedules: fixed 32-node groups, per-group block counts padded
    to the max over cores (one SPMD module). Pure numpy indexing."""
    src = np.asarray(edge_index[0], dtype=np.int64)
    dst = np.asarray(edge_index[1], dtype=np.int64)
    loop = np.arange(n_all, dtype=np.int64)
    src = np.concatenate([src, loop])
    dst = np.concatenate([dst, loop])
    order = np.argsort(dst, kind="stable")
    src = src[order]
    dst = dst[order]

    n_grp = (npc + WIN - 1) // WIN
    g_pad = ((n_grp + GPB - 1) // GPB) * GPB
    nsb = g_pad // GPB
    pad_n = g_pad * WIN - npc
    core_bounds = np.searchsorted(dst, np.arange(n_cores + 1) * npc)

    # stage A: per core, degree-balanced assignment of nodes to 32-node
    # groups (minimizes per-group block counts AND aligns them across
    # cores so the SPMD max-over-cores padding is tight), then per-group
    # lo/hi edge arrays. Device rows come out in group-slot order; run()
    # un-permutes on the host (indexing only).
    per_cg = []
    perms = []
    for c in range(n_cores):
        b0, b1 = core_bounds[c], core_bounds[c + 1]
        s = src[b0:b1]
        d = (dst[b0:b1] - c * npc).astype(np.int64)
        if pad_n:  # virtual degree-1 edges for pad slots
            s = np.concatenate([s, np.zeros(pad_n, dtype=np.int64)])
            d = np.concatenate([d, np.arange(npc, npc + pad_n, dtype=np.int64)])
        ntot = g_pad * WIN
        lo_m = s < SPLIT
        deg_lo = np.bincount(d[lo_m], minlength=ntot).astype(np.float64)
        deg_hi = np.bincount(d[~lo_m], minlength=ntot).astype(np.float64)
        G = g_pad
        cnt = np.zeros(G, np.int64)
        slo = np.zeros(G, np.int64)
        shi = np.zeros(G, np.int64)
        g_of = np.empty(ntot, np.int64)
        c_of = np.empty(ntot, np.int64)
        dl = deg_lo.astype(np.int64)
        dh = deg_hi.astype(np.int64)
        # greedy bin packing that directly minimizes block-count (ceil)
        # increments; groups end up filled to just under 128-multiples
        for n in np.argsort(-(dl + dh), kind="stable"):
            nlo, nhi = dl[n], dh[n]
            db = (((slo + nlo + BLK - 1) // BLK) - ((slo + BLK - 1) // BLK)
                  + ((shi + nhi + BLK - 1) // BLK) - ((shi + BLK - 1) // BLK))
            # secondary: prefer landing closest to a block boundary
            rem = ((-(slo + nlo)) % BLK) + ((-(shi + nhi)) % BLK)
            score = db * 1024 + (rem >> 3)
            score[cnt >= WIN] = 1 << 30
            g = int(np.argmin(score))
            g_of[n] = g
            c_of[n] = cnt[g]
            cnt[g] += 1
            slo[g] += nlo
            shi[g] += nhi
        # schedule slot k = k-th group by descending block needs (aligns
        # the per-slot maxima across cores)
        gorder = np.lexsort((-shi, -slo,
                             -((slo + BLK - 1) // BLK + (shi + BLK - 1) // BLK)))
        slot_of = np.empty(G, np.int64)
        slot_of[gorder] = np.arange(G)
        eg = slot_of[g_of[d]]                       # edge -> schedule slot
        order2 = np.lexsort((s, eg))                # slot-major, src-sorted
        s2, d2, eg2 = s[order2], d[order2], eg[order2]
        gb = np.searchsorted(eg2, np.arange(G + 1))
        rows = []
        for g in range(G):
            e0, e1 = gb[g], gb[g + 1]
            gs = s2[e0:e1]
            gc = c_of[d2[e0:e1]]
            gdst = np.minimum(c * npc + d2[e0:e1], n_all - 1)
            m = gs < SPLIT
            rows.append(((gs[m], gc[m], gdst[m]),
                         (gs[~m] - SPLIT, gc[~m], gdst[~m])))
        per_cg.append(rows)
        # perm[r]: device row r = slot k*WIN + c -> local node id (or -1)
        perm = np.full(ntot, -1, dtype=np.int64)
        node_rows = slot_of[g_of] * WIN + c_of      # node -> device row
        nodes = np.arange(ntot)
        perm[node_rows] = np.where(nodes < npc, nodes, -1)
        perms.append(perm)

    # stage B: global per-group block counts (max over cores)
    nlo_g = [max((len(per_cg[c][g][0][0]) + BLK - 1) // BLK
                 for c in range(n_cores)) for g in range(g_pad)]
    nhi_g = [max((len(per_cg[c][g][1][0]) + BLK - 1) // BLK
                 for c in range(n_cores)) for g in range(g_pad)]
    sched = []
    for sb in range(nsb):
        gs = range(sb * GPB, (sb + 1) * GPB)
        sched.append(([nlo_g[g] for g in gs], [nhi_g[g] for g in gs]))

    # stage C: per-core padded stream arrays
    def pad_block(vals, nblk, fill, dtype):
        a = np.full(nblk * BLK, fill, dtype=dtype)
        a[:len(vals)] = vals
        return a

    cores = []
    for c in range(n_cores):
        sbs = []
        for sb in range(nsb):
            gl = range(sb * GPB, (sb + 1) * GPB)
            idx_parts, col_parts, src_parts, dst_parts = [], [], [], []
            for half in (0, 1):
                cnt_g = nlo_g if half == 0 else nhi_g
                for g in gl:
                    hs, hc, hd = per_cg[c][g][half]
                    nb = cnt_g[g]
                    if nb == 0:
                        continue
                    idx_parts.append((half, pad_block(hs, nb, 0, np.int64)))
                    col_parts.append(pad_block(hc.astype(np.float16), nb,
                                               DEAD, np.float16))
                    src_parts.append(pad_block(
                        hs + (0 if half == 0 else SPLIT), nb, 0, np.int64))
                    dst_parts.append(pad_block(hd, nb, 0, np.int64))
            lo_idx = np.concatenate([a for h, a in idx_parts if h == 0]) \
                if any(h == 0 for h, _ in idx_parts) else np.zeros(0, np.int64)
            hi_idx = np.concatenate([a for h, a in idx_parts if h == 1]) \
                if any(h == 1 for h, _ in idx_parts) else np.zeros(0, np.int64)
            colidx = np.concatenate(col_parts).reshape(-1, BLK)   # [nbk,128]
            srcid = np.concatenate(src_parts).reshape(-1, BLK)
            dstid = np.concatenate(dst_parts).reshape(-1, BLK)
            sbs.append(dict(
                idx_lo=_wrap16(lo_idx) if len(lo_idx) else
                    np.zeros((128, 0), np.int16),
                idx_hi=_wrap16(hi_idx) if len(hi_idx) else
                    np.zeros((128, 0), np.int16),
                colidx=np.ascontiguousarray(colidx.T),            # [128,nbk]
                srcid=srcid, dstid=dstid))
        cores.append(sbs)
    return nsb, sched, cores, perms


def build_streams(cores, statv):
    """Per-edge a_src/a_dst expansion (indexing only) + packed stream blob."""
    asrcv, adstv = statv[:HEADS], statv[HEADS:]             # [4, n_all] f32
    outs = []
    for sbs in cores:
        blobs = []
        for sb in sbs:
            a_s = np.moveaxis(asrcv[:, sb["srcid"]], 0, -1)  # [nbk,128,4]
            a_d = np.moveaxis(adstv[:, sb["dstid"]], 0, -1)
            a8 = np.concatenate([a_s, a_d], axis=2)          # [nbk,128,8]
            a8 = np.ascontiguousarray(
                a8.transpose(1, 0, 2).astype(np.float16))    # [128,nbk,8]
            # colidx replicated WIN x: the on-device one-hot compare then
            # has stride-1 operands only, enabling the DVE 2x f16 mode
            cxe = np.repeat(sb["colidx"][:, :, None], WIN, axis=2)
            blobs.append(np.concatenate(
                [sb["idx_lo"], sb["idx_hi"],
                 cxe.reshape(128, -1).view(np.int16),
                 a8.reshape(128, -1).view(np.int16)], axis=1))
        outs.append(np.ascontiguousarray(np.concatenate(blobs, axis=1)))
    return outs


# -------------------------------------------------------------- device side

def build_nc2(cfg: Cfg, sched):
    nc = bacc.Bacc("TRN2", target_bir_lowering=False, debug=False,
                   num_devices=cfg.n_cores, num_swdge_queues=cfg.nq,
                   dynamic_dma_scratch_size=cfg.scratch)
    npc = cfg.npc
    HD = HEADS * OUT_DIM
    nsb = len(sched)
    nbk_s = [sum(l) + sum(h) for l, h in sched]
    TOT = sum((8 + WIN + 8) * b for b in nbk_s)
    nlo_max = max(sum(l) for l, _ in sched)
    nhi_max = max(sum(h) for _, h in sched)
    nbk_max = max(nbk_s)

    x_t = nc.dram_tensor("x16", [N_NODES, IN_DIM], F16, kind="ExternalInput")
    wf_t = nc.dram_tensor("wf", [IN_DIM, HD], F16, kind="ExternalInput")
    iota_t = nc.dram_tensor("iota16", [128, WIN], F16, kind="ExternalInput")
    biasb_t = nc.dram_tensor("biasb", [128, HD], F32, kind="ExternalInput")
    st_t = nc.dram_tensor("streams", [128, TOT], I16, kind="ExternalInput")
    npad = nsb * GPB * WIN
    out_t = nc.dram_tensor("out", [npad, HD], F16, kind="ExternalOutput")

    with tile.TileContext(nc) as tc:
        with tc.tile_pool(name="consts", bufs=1) as cpool:
            wf_c = cpool.tile([IN_DIM, HD], F16)
            nc.sync.dma_start(wf_c[:], wf_t.ap())
            iota_c = cpool.tile([128, WIN], F16)
            nc.sync.dma_start(iota_c[:], iota_t.ap())
            biasb_c = cpool.tile([128, HD], F32)
            nc.sync.dma_start(biasb_c[:], biasb_t.ap())
            ones_c = cpool.tile([128, 1], F16)
            nc.vector.memset(ones_c[:], 1.0)

            with (tc.tile_pool(name="pb_g", bufs=3) as gp,
                  tc.tile_pool(name="pb_m", bufs=3) as mp,
                  tc.tile_pool(name="pb_s", bufs=3) as sp,
                  tc.tile_pool(name="pb_z", bufs=2) as zp,
                  tc.tile_pool(name="pb_u", bufs=3, space="PSUM") as pu,
                  tc.tile_pool(name="pb_d", bufs=2, space="PSUM") as pdp,
                  tc.tile_pool(name="pb_w", bufs=3, space="PSUM") as pw):
                BL = cfg.blevel
                qi = 0
                for _rep in range(cfg.repeat):
                    off = 0
                    for sb in range(nsb):
                        nlo_l, nhi_l = sched[sb]
                        nlo, nhi = sum(nlo_l), sum(nhi_l)
                        nbk = nlo + nhi
                        W_sb = (8 + WIN + 8) * nbk
                        S0 = 8 * nlo
                        S1 = 8 * nbk
                        S2 = S1 + WIN * nbk
                        strm = sp.tile([128, (8 + WIN + 8) * nbk_max], I16,
                                       tag="strm")
                        nc.sync.dma_start(strm[:, :W_sb],
                                          st_t.ap()[:, off:off + W_sb])
                        off += W_sb
                        il = strm[:, 0:S0]
                        ih = strm[:, S0:S1]
                        cx = (strm[:, S1:S2].bitcast(F16)
                              .rearrange("p (b c) -> p b c", c=WIN))
                        a8 = (strm[:, S2:W_sb].bitcast(F16)
                              .rearrange("p (b k) -> p b k", k=8))

                        glo = gp.tile([128, nlo_max, IN_DIM], F16, tag="glo")
                        for j0, nj in _gather_chunks(nlo * BLK, cfg.gnj):
                            nc.gpsimd.dma_gather(
                                glo[:, j0 // 128:(j0 + nj) // 128, :],
                                x_t.ap()[0:SPLIT, :],
                                il[:, j0 // 16:(j0 + nj) // 16],
                                nj, nj, IN_DIM, single_packet=cfg.sp,
                                queue_num=qi % cfg.nq)
                            qi += 1
                        ghi = gp.tile([128, nhi_max, IN_DIM], F16, tag="ghi")
                        for j0, nj in _gather_chunks(nhi * BLK, cfg.gnj):
                            nc.gpsimd.dma_gather(
                                ghi[:, j0 // 128:(j0 + nj) // 128, :],
                                x_t.ap()[SPLIT:N_NODES, :],
                                ih[:, j0 // 16:(j0 + nj) // 16],
                                nj, nj, IN_DIM, single_packet=cfg.sp,
                                queue_num=qi % cfg.nq)
                            qi += 1

                        if BL < 2:
                            continue
                        # expa = exp(lrelu(a_src + a_dst)), all f16
                        asum = sp.tile([128, nbk_max, HEADS], F16, tag="asum")
                        nc.vector.tensor_tensor(out=asum[:, :nbk, :],
                                                in0=a8[:, :, 0:HEADS],
                                                in1=a8[:, :, HEADS:8],
                                                op=mybir.AluOpType.add)
                        alr = sp.tile([128, nbk_max, HEADS], F16, tag="alr")
                        nc.vector.scalar_tensor_tensor(
                            out=alr[:, :nbk, :], in0=asum[:, :nbk, :],
                            scalar=LEAKY_SLOPE, in1=asum[:, :nbk, :],
                            op0=mybir.AluOpType.mult, op1=mybir.AluOpType.max)
                        expa = sp.tile([128, nbk_max, HEADS], F16, tag="expa")
                        nc.scalar.activation(expa[:, :nbk, :], alr[:, :nbk, :],
                                             mybir.ActivationFunctionType.Exp)
                        # onehot[e, b, c] = (iota[c] == colidx[e, b])
                        # Mw[e, b, h*32+c] = oneh * expa
                        # (built in two block-range chunks: head on DVE,
                        # tail on the otherwise-idle GPSIMD Q7 cores)
                        oneh = mp.tile([128, nbk_max, WIN], F16, tag="oneh")
                        mw = mp.tile([128, nbk_max, HEADS, WIN], F16, tag="mw")
                        nc.vector.tensor_tensor(
                            out=oneh[:, :nbk, :],
                            in0=iota_c[:].unsqueeze(1)
                                .to_broadcast([128, nbk, WIN]),
                            in1=cx,
                            op=mybir.AluOpType.is_equal)
                        # mw multiply split: head on DVE, tail on the
                        # otherwise-idle GPSIMD (is_equal is DVE-only)
                        nsp = nbk - (nbk * cfg.poolpct) // 100
                        for b0, b1, eng in ((0, nsp, nc.vector),
                                            (nsp, nbk, nc.gpsimd)):
                            if b0 == b1:
                                continue
                            nb = b1 - b0
                            eng.tensor_tensor(
                                out=mw[:, b0:b1, :, :],
                                in0=oneh[:, b0:b1, :].unsqueeze(2)
                                    .to_broadcast([128, nb, HEADS, WIN]),
                                in1=expa[:, b0:b1, :].unsqueeze(3)
                                    .to_broadcast([128, nb, HEADS, WIN]),
                                op=mybir.AluOpType.mult)

                        if BL < 3:
                            continue
                        dt_ps = pdp.tile([128, GPB], F32, tag="dt")
                        zall_ps = pw.tile([WIN, GPB, HEADS, OUT_DIM], F32,
                                          tag="zall")
                        lo_c = np.cumsum([0] + nlo_l)
                        hi_c = np.cumsum([0] + nhi_l)
                        for g in range(GPB):
                            blocks = (
                                [(glo, lo_c[g] + j, lo_c[g] + j)
                                 for j in range(nlo_l[g])]
                                + [(ghi, hi_c[g] + j, nlo + hi_c[g] + j)
                                   for j in range(nhi_l[g])])
                            u2 = pu.tile([128, HD], F32, tag="u2")
                            for k, (gt, slot, bcol) in enumerate(blocks):
                                st = k == 0
                                sp_ = k == len(blocks) - 1
                                nc.tensor.matmul(u2[:],
                                                 lhsT=gt[:, slot, :],
                                                 rhs=mw[:, bcol, :, :],
                                                 start=st, stop=sp_)
                                nc.tensor.matmul(dt_ps[:, g:g + 1],
                                                 lhsT=mw[:, bcol, :, :],
                                                 rhs=ones_c[:],
                                                 start=st, stop=sp_)
                            if BL < 4:
                                continue
                            u2s = zp.tile([128, HD], F16, tag="u2s")
                            nc.scalar.activation(
                                u2s[:], u2[:],
                                mybir.ActivationFunctionType.Copy)
                            for h in range(HEADS):
                                nc.tensor.matmul(
                                    zall_ps[:, g, h, :],
                                    lhsT=u2s[:, h * WIN:(h + 1) * WIN],
                                    rhs=wf_c[:, h * OUT_DIM:(h + 1) * OUT_DIM],
                                    start=True, stop=True)
                        if BL < 4:
                            continue
                        # denominators -> [c, g, h]; normalize; swish; store
                        dts = zp.tile([128, GPB], F32, tag="dts")
                        nc.scalar.copy(dts[:], dt_ps[:])
                        rec = zp.tile([WIN, GPB, HEADS], F32, tag="rec")
                        for h in range(HEADS):
                            nc.scalar.copy(
                                rec[:, :, h], dts[h * WIN:(h + 1) * WIN, :])
                        nc.vector.reciprocal(rec[:], rec[:])
                        zn = zp.tile([WIN, GPB, HEADS, OUT_DIM], F16, tag="zn")
                        nc.vector.tensor_tensor(
                            out=zn[:], in0=zall_ps[:],
                            in1=rec[:].unsqueeze(3)
                                .to_broadcast([WIN, GPB, HEADS, OUT_DIM]),
                            op=mybir.AluOpType.mult)
                        if cfg.bias_nonzero:
                            nc.vector.tensor_tensor(
                                out=zn[:], in0=zn[:],
                                in1=biasb_c[0:WIN, :]
                                    .rearrange("c (h o) -> c h o", o=OUT_DIM)
                                    .unsqueeze(1)
                                    .to_broadcast([WIN, GPB, HEADS, OUT_DIM]),
                                op=mybir.AluOpType.add)
                        # flat 2D APs: 4D inner-row walking costs ~70ns/row
                        znf = zn[:].rearrange("c g h o -> c (g h o)")
                        sg = zp.tile([WIN, GPB * HD], F16, tag="sg")
                        nc.scalar.activation(
                            sg[:], znf, mybir.ActivationFunctionType.Sigmoid)
                        mix = zp.tile([WIN, GPB * HD], F16, tag="mix")
                        nc.vector.tensor_scalar(mix[:], sg[:], CMIX - BETA,
                                                BETA, mybir.AluOpType.mult,
                                                mybir.AluOpType.add)
                        zrow = zp.tile([WIN, GPB, HEADS, OUT_DIM], F16,
                                       tag="zrow")
                        nc.vector.tensor_tensor(
                            out=zrow[:].rearrange("c g h o -> c (g h o)"),
                            in0=znf, in1=mix[:],
                            op=mybir.AluOpType.mult)
                        n0 = sb * GPB * WIN
                        nc.sync.dma_start(
                            out_t.ap()[n0:n0 + GPB * WIN, :]
                                 .rearrange("(g c) (h o) -> c g h o",
                                            c=WIN, o=OUT_DIM),
                            zrow[:])
    nc.compile()
    return nc


# ---------------------------------------------------------------- the API

def run(x, edge_index, W, att_src, att_dst, bias, trace=False, cfg_kw=None):
    npc = N_NODES // N_CORES
    nsb, sched, cores, perms = preprocess(edge_index, N_NODES, npc, N_CORES)

    x16 = np.asarray(x, np.float32).astype(np.float16)      # [N, 128]
    W32 = np.asarray(W, dtype=np.float32)
    as32 = np.asarray(att_src, dtype=np.float32)
    ad32 = np.asarray(att_dst, dtype=np.float32)
    bias32 = np.asarray(bias, dtype=np.float32)
    S = np.zeros((HEADS * OUT_DIM, 2 * HEADS), dtype=np.float32)
    for h in range(HEADS):
        S[h * OUT_DIM:(h + 1) * OUT_DIM, h] = as32[h]
        S[h * OUT_DIM:(h + 1) * OUT_DIM, HEADS + h] = ad32[h]
    wad = (W32 @ S).astype(np.float16)          # param-only host matmul

    # launch 1: per-node attention stats
    nc1 = build_nc_stats(npc, N_CORES)
    in_maps1 = [dict(
        xT_slab=np.ascontiguousarray(x16[c * npc:(c + 1) * npc].T),
        wad_pd=wad) for c in range(N_CORES)]
    res1 = run_bass_kernel_spmd(nc1, in_maps1, core_ids=list(range(N_CORES)),
                                trace=trace)
    statv = np.concatenate(
        [res1.results[c]["statv"] for c in range(N_CORES)], axis=1)

    streams = build_streams(cores, statv)
    bias_nonzero = bool(np.any(bias32))
    kw = dict(cfg_kw or {})
    cfg = Cfg(npc, n_cores=N_CORES, bias_nonzero=bias_nonzero, **kw)

    nc2 = build_nc2(cfg, sched)
    iota = np.tile(np.arange(WIN, dtype=np.float16), (128, 1))
    biasb = np.tile(bias32, (128, 1)).astype(np.float32)
    wf16 = W32.astype(np.float16)
    in_maps = [dict(x16=x16, wf=wf16, iota16=iota, biasb=biasb,
                    streams=streams[c]) for c in range(N_CORES)]
    res = run_bass_kernel_spmd(nc2, in_maps, core_ids=list(range(N_CORES)),
                               trace=trace)
    out = np.empty((N_NODES, HEADS * OUT_DIM), np.float32)
    for c in range(N_CORES):
        dev = res.results[c]["out"]                 # [npad, HD] slot order
        perm = perms[c]
        valid = perm >= 0
        out[c * npc + perm[valid]] = dev[valid]
    parts = dict(nc1=nc1, in_maps1=in_maps1, nc2=nc2, in_maps2=in_maps,
                 res1=res1, res2=res, n_cores=N_CORES, cfg=cfg, npc=npc,
                 sched=sched)
    return out, parts


def make_pjrt_fn(nc, in_maps, n_cores):
    """Build a jitted PJRT executor for a prebuilt Bass module (axon path).
    Returns (fn, args); inputs are pre-staged on device."""
    import jax
    from jax.sharding import Mesh, NamedSharding, PartitionSpec
    from jax.experimental.shard_map import shard_map

    import concourse.mybir as mybir_
    from concourse import bass2jax as b2j

    b2j.install_neuronx_cc_hook()
    partition_name = (nc.partition_id_tensor.name
                      if nc.partition_id_tensor else None)
    in_names, out_names, out_avals, zero_outs = [], [], [], []
    for alloc in nc.m.functions[0].allocations:
        if not isinstance(alloc, mybir_.MemoryLocationSet):
            continue
        name = alloc.memorylocations[0].name
        if alloc.kind == "ExternalInput":
            if name != partition_name:
                in_names.append(name)
        elif alloc.kind == "ExternalOutput":
            dt = mybir_.dt.np(alloc.dtype)
            out_avals.append(jax.core.ShapedArray(tuple(alloc.tensor_shape), dt))
            out_names.append(name)
            zero_outs.append(np.zeros(tuple(alloc.tensor_shape), dt))

    # the bind's in_names must cover ALL operands (inputs + zero-out bufs
    # + partition id) — neuronx_cc_hook asserts len(in_names) == n_operands.
    bind_names = list(in_names) + list(out_names)
    if partition_name is not None:
        bind_names.append(partition_name)

    def _body(*args):
        operands = list(args)
        if partition_name is not None:
            operands.append(b2j.partition_id_tensor())
        outs = b2j._bass_exec_p.bind(
            *operands, out_avals=tuple(out_avals), in_names=tuple(bind_names),
            out_names=tuple(out_names), lowering_input_output_aliases=(),
            sim_require_finite=True, sim_require_nnan=True, nc=nc)
        return tuple(outs)

    n_params = len(in_names)
    devices = jax.devices()[:n_cores]
    mesh = Mesh(np.asarray(devices), ("core",))
    spec = PartitionSpec("core")
    fn = jax.jit(shard_map(_body, mesh=mesh,
                           in_specs=(spec,) * (n_params + len(zero_outs)),
                           out_specs=(spec,) * len(out_names),
                           check_rep=False), keep_unused=True)
    sh = NamedSharding(mesh, spec)
    args = [jax.device_put(
                np.concatenate([in_maps[c][nm] for c in range(n_cores)], 0), sh)
            for nm in in_names]
    args += [jax.device_put(
                np.zeros((n_cores * z.shape[0], *z.shape[1:]), z.dtype), sh)
             for z in zero_outs]
    return fn, args


def bench_pair(fnA, argsA, fnB, argsB, iters=24):
    """Interleaved wall-clock of two executables with a 4-byte D2H fetch as
    the completion sync (block_until_ready alone is lost in ~40-90ms axon RPC
    noise; interleaving + cluster-min cancels the shared offset)."""
    import time as _time
    for fn, args in ((fnA, argsA), (fnB, argsB)):
        r = fn(*args)
        _ = np.asarray(r[0][0:1, 0:1])
    pa, pb = [], []
    for _ in range(iters):
        t0 = _time.perf_counter()
        r = fnA(*argsA)
        _ = np.asarray(r[0][0:1, 0:1])
        pa.append(_time.perf_counter() - t0)
        t0 = _time.perf_counter()
        r = fnB(*argsB)
        _ = np.asarray(r[0][0:1, 0:1])
        pb.append(_time.perf_counter() - t0)
    return np.array(pa), np.array(pb)


def bench_slope(ncA, ncB, in_maps, n_cores, reps, iters=24):
    """HW ns of one kernel body via the repeat-slope method: ncA has repeat=1,
    ncB has repeat=reps; returns (per-rep seconds, raw pair arrays)."""
    fnA, argsA = make_pjrt_fn(ncA, in_maps, n_cores)
    fnB, argsB = make_pjrt_fn(ncB, in_maps, n_cores)
    pa, pb = bench_pair(fnA, argsA, fnB, argsB, iters=iters)
    # medians: robust against the occasional anomalous fast RPC round-trip
    per = (np.median(pb) - np.median(pa)) / (reps - 1)
    return max(0.0, per), (pa, pb)


def kernel(**inputs) -> np.ndarray:
    out, _ = run(inputs["x"], inputs["edge_index"], inputs["W"],
                 inputs["att_src"], inputs["att_dst"], inputs["bias"])
    return out
